# revision 12
# baseline (speedup 1.0000x reference)
"""Trainium2 Bass kernel for a pre-norm transformer decoder layer (fp32).

kernel(**inputs) takes the full unsharded inputs of reference.setup_inputs()
and returns the full [2, 2048, 1024] fp32 output.

Sharding: 8 NeuronCores, token-parallel, zero collectives. Core i handles
batch b = i // 4 and query chunk c = i % 4 (512 tokens). Each core computes
the full-batch K/V projections it needs locally (SA K/V from LN1(x) of its
batch; CA K/V from the raw encoder output). The token axis is rolled per
core so its own query chunk sits at positions [0:512) -- one SPMD program,
per-core data only.

On-device layout is feature-major (xT [D, T]): weights load as lhsT with the
contraction on partitions and no activation transposes are needed anywhere.
LayerNorm stats use ones-column matmuls (partition reductions on the tensor
engine), softmax runs without max-subtraction (scores are O(1) here by
construction), the mask is a multiplicative 0/1 fp32 mask applied after exp
(exact for any mask content), and the softmax denominator falls out of an
appended ones-column in V. gamma/beta/attention-scale are folded into the
weights on the host. K^T and V are spilled to DRAM scratch and streamed
back per head during attention.
"""

import sys
sys.path.insert(0, "/opt/trn_rl_repo")

import numpy as np

D = 1024
H = 16
DK = 64
DFF = 4096
S = 2048
CH = 512
EPS = 1e-6
CT = D // 128    # 8 feature tiles
TT = S // 128    # 16 token tiles
FT = DFF // 128  # 32 ff tiles
NG = 4           # score k-tiles per exp/mask group

_CACHE = {}


def _build(apply_src_mask: bool):
    import concourse.bacc as bacc
    import concourse.tile as tile
    from concourse import mybir

    F32 = mybir.dt.float32
    AF = mybir.ActivationFunctionType
    OP = mybir.AluOpType

    nc = bacc.Bacc("TRN2", target_bir_lowering=False, debug=False)

    xT = nc.dram_tensor("xT", [D, S], F32, kind="ExternalInput")
    encT = nc.dram_tensor("encT", [D, S], F32, kind="ExternalInput")
    maskT_d = nc.dram_tensor("maskT", [S, CH], F32, kind="ExternalInput")
    maskc_d = None
    if apply_src_mask:
        maskc_d = nc.dram_tensor("maskc", [S, 1], F32, kind="ExternalInput")
    w_d = {}
    for nm in ("sa_wq", "sa_wk", "sa_wv", "sa_wo", "ca_wq", "ca_wk", "ca_wv", "ca_wo"):
        w_d[nm] = nc.dram_tensor(nm, [D, D], F32, kind="ExternalInput")
    w_d["ff_w1"] = nc.dram_tensor("ff_w1", [D, DFF], F32, kind="ExternalInput")
    w_d["ff_w2"] = nc.dram_tensor("ff_w2", [DFF, D], F32, kind="ExternalInput")
    bc_d = {}  # bias columns [128, CT]
    for nm in ("sa_bq", "sa_bk", "sa_bo", "ca_bq", "ca_bk", "ca_bo", "ff_b2"):
        bc_d[nm] = nc.dram_tensor(nm, [128, CT], F32, kind="ExternalInput")
    br_d = {}  # bias rows [1, D] (for rank-1 adds on token-major outputs)
    for nm in ("sa_bv", "ca_bv"):
        br_d[nm] = nc.dram_tensor(nm, [1, D], F32, kind="ExternalInput")
    fb1_d = nc.dram_tensor("ff_b1", [128, FT], F32, kind="ExternalInput")
    ones_r_d = nc.dram_tensor("ones_r", [1, 128], F32, kind="ExternalInput")
    ones_c_d = nc.dram_tensor("ones_c", [128, 1], F32, kind="ExternalInput")
    yT = nc.dram_tensor("yT", [D, CH], F32, kind="ExternalOutput")

    with tile.TileContext(nc) as tc:
        with (
            tc.tile_pool(name="const", bufs=1) as constp,
            tc.tile_pool(name="ev", bufs=4) as ev,
            tc.tile_pool(name="dram", bufs=1, space="DRAM") as dram,
        ):
            t_ones_r = constp.tile([1, 128], F32, tag="ones_r")
            nc.sync.dma_start(t_ones_r[:], ones_r_d[:])
            t_ones_sq = constp.tile([128, 128], F32, tag="ones_sq")
            nc.vector.memset(t_ones_sq[:], 1.0)
            t_ones_c = constp.tile([128, 1], F32, tag="ones_c")
            nc.sync.dma_start(t_ones_c[:], ones_c_d[:])
            t_bc = {}
            for nm, hnd in bc_d.items():
                t_bc[nm] = constp.tile([128, CT], F32, tag=f"b_{nm}", name=f"b_{nm}")
                nc.sync.dma_start(t_bc[nm][:], hnd[:])
            t_br = {}
            for nm, hnd in br_d.items():
                t_br[nm] = constp.tile([1, D], F32, tag=f"b_{nm}", name=f"b_{nm}")
                nc.sync.dma_start(t_br[nm][:], hnd[:])
            t_fb1 = constp.tile([128, FT], F32, tag="b_ff_b1")
            nc.sync.dma_start(t_fb1[:], fb1_d[:])
            t_maskc = None
            if apply_src_mask:
                t_maskc = constp.tile([128, TT], F32, tag="maskc")
                nc.sync.dma_start(t_maskc[:],
                                  maskc_d.rearrange("(t p) o -> p (t o)", p=128))

            # DRAM scratch
            k_sa_scr = dram.tile([D, S], F32, tag="k_sa")
            v_sa_scr = dram.tile([S, D], F32, tag="v_sa")
            k_ca_scr = dram.tile([D, S], F32, tag="k_ca")
            v_ca_scr = dram.tile([S, D], F32, tag="v_ca")
            x1_scr = dram.tile([D, CH], F32, tag="x1")
            x2_scr = dram.tile([D, CH], F32, tag="x2")

            # ------------------------------------------------------------
            # helpers (each opens its own scoped pools)
            # ------------------------------------------------------------

            def ln_stats(src_dram, ntok, rows_pool):
                """Feature-major DRAM src [D, ntok] -> (mean, rstd) lists of
                [1, 512] SBUF tiles (ntok//512 chunks) in rows_pool."""
                nch = ntok // 512
                mean = [rows_pool.tile([1, 512], F32, tag=f"mean{i}", name=f"mean{i}") for i in range(nch)]
                rstd = [rows_pool.tile([1, 512], F32, tag=f"rstd{i}", name=f"rstd{i}") for i in range(nch)]
                with (
                    tc.tile_pool(name="lns", bufs=2) as sp,
                    tc.tile_pool(name="lnp", bufs=1, space="PSUM") as pp,
                ):
                    s1c = [pp.tile([1, 512], F32, tag=f"s1_{i}", name=f"s1_{i}") for i in range(nch)]
                    s2c = [pp.tile([1, 512], F32, tag=f"s2_{i}", name=f"s2_{i}") for i in range(nch)]
                    for c in range(CT):
                        xc = sp.tile([128, ntok], F32, tag="xs")
                        nc.sync.dma_start(xc[:], src_dram[c * 128:(c + 1) * 128, :])
                        xsq = sp.tile([128, ntok], F32, tag="sq")
                        nc.scalar.activation(xsq[:], xc[:], AF.Square)
                        for ch in range(nch):
                            sl = slice(ch * 512, (ch + 1) * 512)
                            nc.tensor.matmul(s1c[ch][:], t_ones_c[:], xc[:, sl],
                                             start=(c == 0), stop=(c == CT - 1))
                            nc.tensor.matmul(s2c[ch][:], t_ones_c[:], xsq[:, sl],
                                             start=(c == 0), stop=(c == CT - 1))
                    for ch in range(nch):
                        t1 = sp.tile([1, 512], F32, tag="t1")
                        t2 = sp.tile([1, 512], F32, tag="t2")
                        nc.vector.tensor_scalar_mul(mean[ch][:], s1c[ch][:], 1.0 / D)
                        nc.vector.tensor_mul(t1[:], s1c[ch][:], mean[ch][:])
                        nc.vector.tensor_sub(t1[:], s2c[ch][:], t1[:])
                        nc.vector.tensor_scalar_mul(t1[:], t1[:], 1.0 / (D - 1))
                        nc.scalar.activation(t2[:], t1[:], AF.Sqrt)
                        nc.vector.tensor_scalar_add(t2[:], t2[:], EPS)
                        nc.vector.reciprocal(rstd[ch][:], t2[:])
                return mean, rstd

            def proj_block(h, sp, pp, wp, *, half, ntok_half, w_list):
                """Run projections for one token half given normalized (or raw)
                activations h [128, CT, ntok_half].

                w_list: list of dicts with keys:
                  kind: 'kT' (features out, rhs=h) or 'v' (tokens out, lhsT=h)
                  w: dram weight [D, D]; out: for kT -> (scr, bias_col);
                  for v -> (scr, bias_row)
                  qonly: if True, only token range [0:512) of half 0
                """
                base = half * ntok_half
                for spec in w_list:
                    if spec.get("qonly") and half != 0:
                        continue
                    wd = spec["w"]
                    if spec["kind"] == "kT":
                        nch = 1 if spec.get("qonly") else ntok_half // 512
                        for dh in range(2):
                            strips = []
                            for c in range(CT):
                                t = wp.tile([128, 512], F32, tag="wstr", name="wstr")
                                nc.sync.dma_start(
                                    t[:], wd[c * 128:(c + 1) * 128,
                                             dh * 512:(dh + 1) * 512])
                                strips.append(t)
                            for dq in range(4):
                                d = dh * 4 + dq
                                for ch in range(nch):
                                    sl = slice(ch * 512, (ch + 1) * 512)
                                    acc = pp.tile([128, 512], F32, tag="mm")
                                    for c in range(CT):
                                        nc.tensor.matmul(
                                            acc[:],
                                            strips[c][:, dq * 128:(dq + 1) * 128],
                                            h[:, c, sl],
                                            start=(c == 0), stop=(c == CT - 1))
                                    if spec.get("qonly"):
                                        qdst = spec["out"]
                                        nc.vector.tensor_scalar(
                                            qdst[:, d, :], acc[:],
                                            spec["bias"][:, d:d + 1], None, OP.add)
                                    else:
                                        kt_scr = spec["out"]
                                        ot = sp.tile([128, 512], F32, tag="kev")
                                        nc.vector.tensor_scalar(
                                            ot[:], acc[:],
                                            spec["bias"][:, d:d + 1], None, OP.add)
                                        nc.sync.dma_start(
                                            kt_scr[d * 128:(d + 1) * 128,
                                                   base + ch * 512:base + (ch + 1) * 512],
                                            ot[:])
                    else:  # 'v'
                        v_scr = spec["out"]
                        brow = spec["bias"]
                        for dvc in range(2):
                            strips = []
                            for c in range(CT):
                                t = wp.tile([128, 512], F32, tag="wstr", name="wstr")
                                nc.sync.dma_start(
                                    t[:], wd[c * 128:(c + 1) * 128,
                                             dvc * 512:(dvc + 1) * 512])
                                strips.append(t)
                            for tt in range(ntok_half // 128):
                                acc = pp.tile([128, 512], F32, tag="mm")
                                nc.tensor.matmul(acc[:], t_ones_r[:],
                                                 brow[:, dvc * 512:(dvc + 1) * 512],
                                                 start=True, stop=False)
                                for c in range(CT):
                                    nc.tensor.matmul(
                                        acc[:], h[:, c, tt * 128:(tt + 1) * 128],
                                        strips[c][:, :], start=False,
                                        stop=(c == CT - 1))
                                ot = sp.tile([128, 512], F32, tag="vev")
                                nc.vector.tensor_copy(ot[:], acc[:])
                                nc.sync.dma_start(
                                    v_scr[base + tt * 128:base + (tt + 1) * 128,
                                          dvc * 512:(dvc + 1) * 512],
                                    ot[:])

            def attention(q, k_scr, v_scr, mask_tile, use_maskc, O, sp, pp):
                """q [128, CT, 512] SBUF; K/V streamed from DRAM scratch.
                Writes O [64, H, 512] (softmax-normalized per head)."""
                kpair = None
                for h in range(H):
                    dt, pr = h // 2, 64 * (h % 2)
                    if pr == 0:
                        kpair = sp.tile([128, S], F32, tag="kstr", bufs=2)
                        nc.sync.dma_start(kpair[:],
                                          k_scr[dt * 128:(dt + 1) * 128, :])
                    grps = []
                    for g in range(TT // NG):
                        att = sp.tile([128, NG, 512], F32, tag="att")
                        for j in range(NG):
                            kt = g * NG + j
                            sps = pp.tile([128, 512], F32, tag="sc")
                            nc.tensor.matmul(sps[:],
                                             kpair[pr:pr + 64, kt * 128:(kt + 1) * 128],
                                             q[pr:pr + 64, dt, :],
                                             start=True, stop=True)
                            nc.scalar.activation(att[:, j, :], sps[:], AF.Exp)
                        if mask_tile is not None:
                            nc.vector.tensor_mul(att[:], att[:],
                                                 mask_tile[:, g * NG:(g + 1) * NG, :])
                        if use_maskc:
                            for j in range(NG):
                                kt = g * NG + j
                                nc.vector.tensor_scalar(
                                    att[:, j, :], att[:, j, :],
                                    t_maskc[:, kt:kt + 1], None, OP.mult)
                        grps.append(att)
                    avp = pp.tile([65, 512], F32, tag="av")
                    for kt in range(TT):
                        va = sp.tile([128, 65], F32, tag="va")
                        nc.sync.dma_start(va[:, 0:64],
                                          v_scr[kt * 128:(kt + 1) * 128,
                                                h * 64:(h + 1) * 64])
                        nc.gpsimd.memset(va[:, 64:65], 1.0)
                        nc.tensor.matmul(avp[:], va[:], grps[kt // NG][:, kt % NG, :],
                                         start=(kt == 0), stop=(kt == TT - 1))
                    rr = sp.tile([65, 512], F32, tag="rr")
                    nc.vector.reciprocal(rr[64:65, :], avp[64:65, :])
                    rbp = pp.tile([64, 512], F32, tag="rb")
                    nc.tensor.matmul(rbp[:], t_ones_sq[64:65, 0:64], rr[64:65, :],
                                     start=True, stop=True)
                    rb = sp.tile([64, 512], F32, tag="rbs")
                    nc.vector.tensor_copy(rb[:], rbp[:])
                    nc.vector.tensor_mul(O[:, h, :], avp[0:64, :], rb[:])

            def out_proj(O, wo_dram, bias_tile, resid_dram, out_dram, sp, pp, wp):
                """out = wo.T @ O + bias_col + resid, streamed to out_dram.

                wo strips are loaded per head at partition base 0 so the lhsT
                base matches the O rhs base (matmul requires equal bases)."""
                for oh in range(2):
                    strips = []
                    for h in range(H):
                        t = wp.tile([64, 512], F32, tag="wstr", name="wstr", bufs=16)
                        nc.sync.dma_start(t[:], wo_dram[h * 64:(h + 1) * 64,
                                                        oh * 512:(oh + 1) * 512])
                        strips.append(t)
                    for oq in range(4):
                        o = oh * 4 + oq
                        acc = pp.tile([128, 512], F32, tag="mm")
                        for h in range(H):
                            nc.tensor.matmul(
                                acc[:],
                                strips[h][:, oq * 128:(oq + 1) * 128],
                                O[:, h, :], start=(h == 0), stop=(h == H - 1))
                        res = sp.tile([128, 512], F32, tag="res")
                        nc.sync.dma_start(res[:],
                                          resid_dram[o * 128:(o + 1) * 128, 0:512])
                        ot = sp.tile([128, 512], F32, tag="xout")
                        nc.vector.scalar_tensor_tensor(ot[:], acc[:],
                                                       bias_tile[:, o:o + 1],
                                                       res[:], OP.add, OP.add)
                        nc.sync.dma_start(out_dram[o * 128:(o + 1) * 128, :], ot[:])

            def attn_block(src_dram, normalize_src, q_w, q_b, k_w, k_b, v_w, v_br,
                           o_w, o_b, k_scr, v_scr, mask_tile_src, use_maskc,
                           resid_dram, out_dram, q_src_dram):
                """One full attention block.
                src_dram: [D, S] K/V source (xT for SA, encT for CA).
                normalize_src: LN the K/V source (True for SA, False for CA).
                q_src_dram: [D, >=512] LN'd source for Q (own chunk);
                  for SA it equals the normalized src (handled inline);
                  for CA pass x1_scr (LN2 applied here)."""
                with tc.tile_pool(name="qkeep", bufs=1) as qkeep:
                    q = qkeep.tile([128, CT, 512], F32, tag="q")
                    with tc.tile_pool(name="rows", bufs=1) as rows_pool:
                        if normalize_src:
                            mean, rstd = ln_stats(src_dram, S, rows_pool)
                        else:
                            mean, rstd = ln_stats(q_src_dram, CH, rows_pool)
                        with (
                            tc.tile_pool(name="prep", bufs=2) as sp,
                            tc.tile_pool(name="wstr", bufs=8) as wp,
                            tc.tile_pool(name="prepp", bufs=2, space="PSUM") as pp,
                        ):
                            with tc.tile_pool(name="hbuf", bufs=1) as hp:
                                if normalize_src:
                                    # SA: normalize src by halves; Q from half 0
                                    h = hp.tile([128, CT, 1024], F32, tag="h")
                                    for half in range(2):
                                        base = half * 1024
                                        for ch2 in range(2):
                                            chg = half * 2 + ch2
                                            sl = slice(ch2 * 512, (ch2 + 1) * 512)
                                            mb = pp.tile([128, 512], F32, tag="mb")
                                            nc.tensor.matmul(mb[:], t_ones_r[:],
                                                             mean[chg][:],
                                                             start=True, stop=True)
                                            rbb = pp.tile([128, 512], F32, tag="rbb")
                                            nc.tensor.matmul(rbb[:], t_ones_r[:],
                                                             rstd[chg][:],
                                                             start=True, stop=True)
                                            for c in range(CT):
                                                xc = sp.tile([128, 512], F32, tag="xs2")
                                                nc.sync.dma_start(
                                                    xc[:],
                                                    src_dram[c * 128:(c + 1) * 128,
                                                             base + ch2 * 512:
                                                             base + (ch2 + 1) * 512])
                                                nc.vector.tensor_sub(h[:, c, sl],
                                                                     xc[:], mb[:])
                                                nc.vector.tensor_mul(h[:, c, sl],
                                                                     h[:, c, sl], rbb[:])
                                        w_list = [
                                            {"kind": "kT", "w": q_w, "bias": q_b,
                                             "out": q, "qonly": True},
                                            {"kind": "kT", "w": k_w, "bias": k_b,
                                             "out": k_scr},
                                            {"kind": "v", "w": v_w, "bias": v_br,
                                             "out": v_scr},
                                        ]
                                        proj_block(h, sp, pp, wp, half=half,
                                                   ntok_half=1024, w_list=w_list)
                                else:
                                    # CA: Q from LN2(x1) first (h2 [128, CT, 512])
                                    h2 = hp.tile([128, CT, 1024], F32, tag="h")
                                    mb = pp.tile([128, 512], F32, tag="mb")
                                    nc.tensor.matmul(mb[:], t_ones_r[:], mean[0][:],
                                                     start=True, stop=True)
                                    rbb = pp.tile([128, 512], F32, tag="rbb")
                                    nc.tensor.matmul(rbb[:], t_ones_r[:], rstd[0][:],
                                                     start=True, stop=True)
                                    for c in range(CT):
                                        xc = sp.tile([128, 512], F32, tag="xs2")
                                        nc.sync.dma_start(
                                            xc[:],
                                            q_src_dram[c * 128:(c + 1) * 128, :])
                                        nc.vector.tensor_sub(h2[:, c, 0:512],
                                                             xc[:], mb[:])
                                        nc.vector.tensor_mul(h2[:, c, 0:512],
                                                             h2[:, c, 0:512], rbb[:])
                                    proj_block(h2, sp, pp, wp, half=0, ntok_half=1024,
                                               w_list=[{"kind": "kT", "w": q_w,
                                                        "bias": q_b, "out": q,
                                                        "qonly": True}])
                                    # raw encoder halves for K/V
                                    for half in range(2):
                                        base = half * 1024
                                        henc = hp.tile([128, CT, 1024], F32, tag="h")
                                        for c in range(CT):
                                            nc.sync.dma_start(
                                                henc[:, c, :],
                                                src_dram[c * 128:(c + 1) * 128,
                                                         base:base + 1024])
                                        w_list = [
                                            {"kind": "kT", "w": k_w, "bias": k_b,
                                             "out": k_scr},
                                            {"kind": "v", "w": v_w, "bias": v_br,
                                             "out": v_scr},
                                        ]
                                        proj_block(henc, sp, pp, wp, half=half,
                                                   ntok_half=1024, w_list=w_list)
                    # attention + out-proj
                    with (
                        tc.tile_pool(name="attn_o", bufs=1) as op_,
                    ):
                        O = op_.tile([64, H, 512], F32, tag="O")
                        with (
                            tc.tile_pool(name="attn", bufs=4) as sp,
                            tc.tile_pool(name="attnp", bufs=2, space="PSUM") as pp,
                        ):
                            mask_tile = None
                            if mask_tile_src is not None:
                                with tc.tile_pool(name="maskp", bufs=1) as mp:
                                    mask_tile = mp.tile([128, TT, 512], F32, tag="m")
                                    nc.sync.dma_start(
                                        mask_tile[:],
                                        mask_tile_src.rearrange("(t p) q -> p t q",
                                                                p=128))
                                    attention(q, k_scr, v_scr, mask_tile, False,
                                              O, sp, pp)
                            else:
                                attention(q, k_scr, v_scr, None, use_maskc,
                                          O, sp, pp)
                        with (
                            tc.tile_pool(name="oproj", bufs=2) as sp,
                            tc.tile_pool(name="wstro", bufs=8) as wp,
                            tc.tile_pool(name="oprojp", bufs=2, space="PSUM") as pp,
                        ):
                            out_proj(O, o_w, o_b, resid_dram, out_dram, sp, pp, wp)

            # ================= Block 1: self-attention =================
            attn_block(xT, True, w_d["sa_wq"], t_bc["sa_bq"], w_d["sa_wk"],
                       t_bc["sa_bk"], w_d["sa_wv"], t_br["sa_bv"], w_d["sa_wo"],
                       t_bc["sa_bo"], k_sa_scr, v_sa_scr, maskT_d, False,
                       xT, x1_scr, None)

            # ================= Block 2: cross-attention =================
            attn_block(encT, False, w_d["ca_wq"], t_bc["ca_bq"], w_d["ca_wk"],
                       t_bc["ca_bk"], w_d["ca_wv"], t_br["ca_bv"], w_d["ca_wo"],
                       t_bc["ca_bo"], k_ca_scr, v_ca_scr, None, apply_src_mask,
                       x1_scr, x2_scr, x1_scr)

            # ================= Block 3: FFN =================
            with tc.tile_pool(name="ffrows", bufs=1) as rows_pool:
                mean3, rstd3 = ln_stats(x2_scr, CH, rows_pool)
                with (
                    tc.tile_pool(name="ffsp", bufs=2) as sp,
                    tc.tile_pool(name="ffw", bufs=6) as wp,
                    tc.tile_pool(name="ffbig", bufs=1) as bigp,
                    tc.tile_pool(name="ffpp", bufs=2, space="PSUM") as pp,
                    tc.tile_pool(name="ffacc", bufs=1, space="PSUM") as accp,
                ):
                    h3 = bigp.tile([128, CT, 512], F32, tag="h3")
                    mb = pp.tile([128, 512], F32, tag="mm")
                    nc.tensor.matmul(mb[:], t_ones_r[:], mean3[0][:],
                                     start=True, stop=True)
                    rbb = pp.tile([128, 512], F32, tag="mm")
                    nc.tensor.matmul(rbb[:], t_ones_r[:], rstd3[0][:],
                                     start=True, stop=True)
                    for c in range(CT):
                        xc = sp.tile([128, 512], F32, tag="xs3")
                        nc.sync.dma_start(xc[:], x2_scr[c * 128:(c + 1) * 128, :])
                        nc.vector.tensor_sub(h3[:, c, :], xc[:], mb[:])
                        nc.vector.tensor_mul(h3[:, c, :], h3[:, c, :], rbb[:])
                    g = bigp.tile([128, FT, 512], F32, tag="g")
                    for f in range(FT):
                        acc = pp.tile([128, 512], F32, tag="mm")
                        for c in range(CT):
                            wt = wp.tile([128, 128], F32, tag="w1t")
                            nc.sync.dma_start(
                                wt[:], w_d["ff_w1"][c * 128:(c + 1) * 128,
                                                    f * 128:(f + 1) * 128])
                            nc.tensor.matmul(acc[:], wt[:], h3[:, c, :],
                                             start=(c == 0), stop=(c == CT - 1))
                        nc.scalar.activation(g[:, f, :], acc[:], AF.Relu,
                                             bias=t_fb1[:, f:f + 1])
                    for oh in range(2):
                        accs = [accp.tile([128, 512], F32, tag=f"acc{i}", name=f"acc{i}")
                                for i in range(4)]
                        for f in range(FT):
                            for oq in range(4):
                                o = oh * 4 + oq
                                wt = wp.tile([128, 128], F32, tag="w2t")
                                nc.sync.dma_start(
                                    wt[:], w_d["ff_w2"][f * 128:(f + 1) * 128,
                                                        o * 128:(o + 1) * 128])
                                nc.tensor.matmul(accs[oq][:], wt[:], g[:, f, :],
                                                 start=(f == 0), stop=(f == FT - 1))
                        for oq in range(4):
                            o = oh * 4 + oq
                            res = sp.tile([128, 512], F32, tag="res3")
                            nc.sync.dma_start(res[:],
                                              x2_scr[o * 128:(o + 1) * 128, :])
                            ot = sp.tile([128, 512], F32, tag="yev")
                            nc.vector.scalar_tensor_tensor(
                                ot[:], accs[oq][:], t_bc["ff_b2"][:, o:o + 1],
                                res[:], OP.add, OP.add)
                            nc.sync.dma_start(yT[o * 128:(o + 1) * 128, :], ot[:])

    nc.compile()
    return nc


def _prep_host(inputs):
    """Host-side folds and per-core data prep."""
    f32 = lambda a: np.ascontiguousarray(np.asarray(a, np.float32))
    x = f32(inputs["x"])
    enc = f32(inputs["encoder_output"])
    tgt = np.asarray(inputs["tgt_mask"])[0, 0].astype(np.float32)   # [S, S]
    src = np.asarray(inputs["src_mask"])[0, 0, 0].astype(np.float32)  # [S]
    g1, b1 = f32(inputs["n1_g"]), f32(inputs["n1_b"])
    g2, b2 = f32(inputs["n2_g"]), f32(inputs["n2_b"])
    g3, b3 = f32(inputs["n3_g"]), f32(inputs["n3_b"])
    scale = np.float32(1.0 / np.sqrt(DK))

    w = {}
    w["sa_wq"] = f32((g1[:, None] * f32(inputs["sa_wq"])) * scale)
    sa_bq = (b1 @ f32(inputs["sa_wq"]) + f32(inputs["sa_bq"])) * scale
    w["sa_wk"] = f32(g1[:, None] * f32(inputs["sa_wk"]))
    sa_bk = b1 @ f32(inputs["sa_wk"]) + f32(inputs["sa_bk"])
    w["sa_wv"] = f32(g1[:, None] * f32(inputs["sa_wv"]))
    sa_bv = b1 @ f32(inputs["sa_wv"]) + f32(inputs["sa_bv"])
    w["sa_wo"] = f32(inputs["sa_wo"])
    sa_bo = f32(inputs["sa_bo"])
    w["ca_wq"] = f32((g2[:, None] * f32(inputs["ca_wq"])) * scale)
    ca_bq = (b2 @ f32(inputs["ca_wq"]) + f32(inputs["ca_bq"])) * scale
    w["ca_wk"] = f32(inputs["ca_wk"])
    ca_bk = f32(inputs["ca_bk"])
    w["ca_wv"] = f32(inputs["ca_wv"])
    ca_bv = f32(inputs["ca_bv"])
    w["ca_wo"] = f32(inputs["ca_wo"])
    ca_bo = f32(inputs["ca_bo"])
    w["ff_w1"] = f32(g3[:, None] * f32(inputs["ff_w1"]))
    ff_b1 = b3 @ f32(inputs["ff_w1"]) + f32(inputs["ff_b1"])
    w["ff_w2"] = f32(inputs["ff_w2"])
    ff_b2 = f32(inputs["ff_b2"])

    col = lambda b: np.ascontiguousarray(np.asarray(b, np.float32).reshape(-1, 128).T)
    row = lambda b: np.ascontiguousarray(np.asarray(b, np.float32).reshape(1, -1))
    shared = dict(w)
    shared["sa_bq"] = col(sa_bq)
    shared["sa_bk"] = col(sa_bk)
    shared["sa_bo"] = col(sa_bo)
    shared["ca_bq"] = col(ca_bq)
    shared["ca_bk"] = col(ca_bk)
    shared["ca_bo"] = col(ca_bo)
    shared["ff_b2"] = col(ff_b2)
    shared["sa_bv"] = row(sa_bv)
    shared["ca_bv"] = row(ca_bv)
    shared["ff_b1"] = col(ff_b1)
    shared["ones_r"] = np.ones((1, 128), np.float32)
    shared["ones_c"] = np.ones((128, 1), np.float32)

    apply_src_mask = not bool(np.all(src == 1.0))
    if apply_src_mask:
        shared["maskc"] = np.ascontiguousarray(src.reshape(S, 1))

    in_maps = []
    for core in range(8):
        b, c = core // 4, core % 4
        q0 = c * CH
        perm = np.r_[q0:q0 + CH, 0:q0, q0 + CH:S]
        m = dict(shared)
        m["xT"] = np.ascontiguousarray(x[b].T[:, perm])
        m["encT"] = np.ascontiguousarray(enc[b].T)
        m["maskT"] = np.ascontiguousarray(tgt[q0:q0 + CH, :].T[perm, :])
        in_maps.append(m)
    return in_maps, apply_src_mask


def kernel(**inputs):
    from concourse.bass_utils import run_bass_kernel_spmd

    in_maps, apply_src_mask = _prep_host(inputs)
    key = apply_src_mask
    if key not in _CACHE:
        _CACHE[key] = _build(apply_src_mask)
    nc = _CACHE[key]
    res = run_bass_kernel_spmd(nc, in_maps, core_ids=list(range(8)))
    out = np.empty((2, S, D), np.float32)
    for core in range(8):
        b, c = core // 4, core % 4
        out[b, c * CH:(c + 1) * CH, :] = res.results[core]["yT"].T
    return out


# revision 16
# speedup vs baseline: 2.4589x; 2.4589x over previous
"""Trainium2 Bass kernel for a pre-norm transformer decoder layer.

kernel(**inputs) takes the full unsharded inputs of reference.setup_inputs()
and returns the full [2, 2048, 1024] fp32 output.

Sharding: 8 NeuronCores, token-parallel, zero collectives. Core i handles
batch b = i // 4 and query chunk c = i % 4 (512 tokens). Each core computes
the full-batch K/V projections it needs locally (SA K/V from LN1(x) of its
batch; CA K/V from the raw encoder output). The token axis is rolled per
core so its own query chunk sits at positions [0:512) -- one SPMD program,
per-core data only.

Numerics: matmul OPERANDS are bf16 (fp32 runs as two PE passes -- half
throughput); accumulation is always fp32 in PSUM. The LayerNorm statistics,
softmax denominators/reciprocals, biases, and the entire residual stream
stay fp32, so rounding error does not compound across blocks.

Layout is feature-major (xT [D, T]): weights load as lhsT with the
contraction on partitions; no activation transposes anywhere. LayerNorm
stats use ones-column matmuls (partition reductions on PE), softmax runs
without max-subtraction (scores are O(1) here by construction), the mask is
multiplicative 0/1 applied after exp (exact for any mask content), and the
softmax denominator falls out of an appended ones-column in V. gamma/beta
and the attention scale are folded into the weights on the host. K^T and V
are spilled to DRAM scratch (bf16) and streamed back per head.
"""

import sys
sys.path.insert(0, "/opt/trn_rl_repo")

import numpy as np

D = 1024
H = 16
DK = 64
DFF = 4096
S = 2048
CH = 512
EPS = 1e-6
CT = D // 128    # 8 feature tiles
TT = S // 128    # 16 token tiles
FT = DFF // 128  # 32 ff tiles
NG = 4           # score k-tiles per exp/mask group

_CACHE = {}


def _build(apply_src_mask: bool):
    import concourse.bacc as bacc
    import concourse.tile as tile
    from concourse import mybir

    F32 = mybir.dt.float32
    BF16 = mybir.dt.bfloat16
    AF = mybir.ActivationFunctionType
    OP = mybir.AluOpType

    nc = bacc.Bacc("TRN2", target_bir_lowering=False, debug=False)

    xT = nc.dram_tensor("xT", [D, S], F32, kind="ExternalInput")
    encT = nc.dram_tensor("encT", [D, S], BF16, kind="ExternalInput")
    maskT_d = nc.dram_tensor("maskT", [S, CH], BF16, kind="ExternalInput")
    maskc_d = None
    if apply_src_mask:
        maskc_d = nc.dram_tensor("maskc", [S, 1], BF16, kind="ExternalInput")
    w_d = {}
    for nm in ("sa_wq", "sa_wk", "sa_wv", "sa_wo", "ca_wq", "ca_wk", "ca_wv", "ca_wo"):
        w_d[nm] = nc.dram_tensor(nm, [D, D], BF16, kind="ExternalInput")
    w_d["ff_w1"] = nc.dram_tensor("ff_w1", [D, DFF], BF16, kind="ExternalInput")
    w_d["ff_w2"] = nc.dram_tensor("ff_w2", [DFF, D], BF16, kind="ExternalInput")
    bc_d = {}  # bias columns [128, CT] fp32
    for nm in ("sa_bq", "sa_bk", "sa_bo", "ca_bq", "ca_bk", "ca_bo", "ff_b2"):
        bc_d[nm] = nc.dram_tensor(nm, [128, CT], F32, kind="ExternalInput")
    br_d = {}  # bias rows [1, D] fp32 (rank-1 adds on token-major outputs)
    for nm in ("sa_bv", "ca_bv"):
        br_d[nm] = nc.dram_tensor(nm, [1, D], F32, kind="ExternalInput")
    fb1_d = nc.dram_tensor("ff_b1", [128, FT], F32, kind="ExternalInput")
    ones_r_d = nc.dram_tensor("ones_r", [1, 128], F32, kind="ExternalInput")
    ones_c_d = nc.dram_tensor("ones_c", [128, 1], F32, kind="ExternalInput")
    yT = nc.dram_tensor("yT", [D, CH], F32, kind="ExternalOutput")

    with tile.TileContext(nc) as tc:
        with (
            tc.tile_pool(name="const", bufs=1) as constp,
            tc.tile_pool(name="dram", bufs=1, space="DRAM") as dram,
        ):
            t_ones_r = constp.tile([1, 128], F32, tag="ones_r")
            nc.sync.dma_start(t_ones_r[:], ones_r_d[:])
            t_ones_sq = constp.tile([128, 128], F32, tag="ones_sq")
            nc.vector.memset(t_ones_sq[:], 1.0)
            t_ones_c = constp.tile([128, 1], F32, tag="ones_c")
            nc.sync.dma_start(t_ones_c[:], ones_c_d[:])
            t_bc = {}
            for nm, hnd in bc_d.items():
                t_bc[nm] = constp.tile([128, CT], F32, tag=f"b_{nm}", name=f"b_{nm}")
                nc.sync.dma_start(t_bc[nm][:], hnd[:])
            t_br = {}
            for nm, hnd in br_d.items():
                t_br[nm] = constp.tile([1, D], F32, tag=f"b_{nm}", name=f"b_{nm}")
                nc.sync.dma_start(t_br[nm][:], hnd[:])
            t_fb1 = constp.tile([128, FT], F32, tag="b_ff_b1")
            nc.sync.dma_start(t_fb1[:], fb1_d[:])
            t_maskc = None
            if apply_src_mask:
                t_maskc = constp.tile([128, TT], BF16, tag="maskc")
                nc.sync.dma_start(t_maskc[:],
                                  maskc_d.rearrange("(t p) o -> p (t o)", p=128))

            # DRAM scratch (K/V in bf16; residual stream fp32)
            k_sa_scr = dram.tile([D, S], BF16, tag="k_sa")
            v_sa_scr = dram.tile([S, D], BF16, tag="v_sa")
            k_ca_scr = dram.tile([D, S], BF16, tag="k_ca")
            v_ca_scr = dram.tile([S, D], BF16, tag="v_ca")
            x1_scr = dram.tile([D, CH], F32, tag="x1")
            x2_scr = dram.tile([D, CH], F32, tag="x2")

            # ------------------------------------------------------------
            # helpers
            # ------------------------------------------------------------

            def ln_stats(src_dram, ntok, rows_pool):
                """Feature-major fp32 DRAM src [D, ntok] -> (mean, rstd) lists
                of [1, 512] fp32 SBUF tiles in rows_pool."""
                nch = ntok // 512
                mean = [rows_pool.tile([1, 512], F32, tag=f"mean{i}", name=f"mean{i}")
                        for i in range(nch)]
                rstd = [rows_pool.tile([1, 512], F32, tag=f"rstd{i}", name=f"rstd{i}")
                        for i in range(nch)]
                with (
                    tc.tile_pool(name="lns", bufs=2) as sp,
                    tc.tile_pool(name="lnp", bufs=1, space="PSUM") as pp,
                ):
                    s1c = [pp.tile([1, 512], F32, tag=f"s1_{i}", name=f"s1_{i}")
                           for i in range(nch)]
                    s2c = [pp.tile([1, 512], F32, tag=f"s2_{i}", name=f"s2_{i}")
                           for i in range(nch)]
                    for c in range(CT):
                        xc = sp.tile([128, ntok], F32, tag="xs")
                        nc.gpsimd.dma_start(xc[:], src_dram[c * 128:(c + 1) * 128, :])
                        xsq = sp.tile([128, ntok], F32, tag="sq")
                        nc.scalar.activation(xsq[:], xc[:], AF.Square)
                        for ch in range(nch):
                            sl = slice(ch * 512, (ch + 1) * 512)
                            nc.tensor.matmul(s1c[ch][:], t_ones_c[:], xc[:, sl],
                                             start=(c == 0), stop=(c == CT - 1))
                            nc.tensor.matmul(s2c[ch][:], t_ones_c[:], xsq[:, sl],
                                             start=(c == 0), stop=(c == CT - 1))
                    for ch in range(nch):
                        t1 = sp.tile([1, 512], F32, tag="t1")
                        t2 = sp.tile([1, 512], F32, tag="t2")
                        nc.vector.tensor_scalar_mul(mean[ch][:], s1c[ch][:], 1.0 / D)
                        nc.vector.tensor_mul(t1[:], s1c[ch][:], mean[ch][:])
                        nc.vector.tensor_sub(t1[:], s2c[ch][:], t1[:])
                        nc.vector.tensor_scalar_mul(t1[:], t1[:], 1.0 / (D - 1))
                        nc.scalar.activation(t2[:], t1[:], AF.Sqrt)
                        nc.vector.tensor_scalar_add(t2[:], t2[:], EPS)
                        nc.vector.reciprocal(rstd[ch][:], t2[:])
                return mean, rstd

            def proj_block(h, sp, pp, wp, *, half, w_list):
                """Projections for one token half. h: bf16 [128, CT, 1024].

                spec kind 'kT': out feature-major (rhs = h, lhsT = weight),
                evict + bias_col -> bf16 scr [D, S] (or SBUF q when qonly).
                spec kind 'v': out token-major (lhsT = h, rhs = weight),
                rank-1 fp32 bias row, evict -> bf16 scr [S, D]."""
                base = half * 1024
                for spec in w_list:
                    if spec.get("qonly") and half != 0:
                        continue
                    wd = spec["w"]
                    if spec["kind"] == "kT":
                        nch = 1 if spec.get("qonly") else 2
                        for dh in range(2):
                            strips = []
                            for c in range(CT):
                                t = wp.tile([128, 512], BF16, tag="wstr", name="wstr")
                                nc.scalar.dma_start(
                                    t[:], wd[c * 128:(c + 1) * 128,
                                             dh * 512:(dh + 1) * 512])
                                strips.append(t)
                            for dq in range(4):
                                d = dh * 4 + dq
                                for ch in range(nch):
                                    sl = slice(ch * 512, (ch + 1) * 512)
                                    acc = pp.tile([128, 512], F32, tag="mm")
                                    for c in range(CT):
                                        nc.tensor.matmul(
                                            acc[:],
                                            strips[c][:, dq * 128:(dq + 1) * 128],
                                            h[:, c, sl],
                                            start=(c == 0), stop=(c == CT - 1))
                                    if spec.get("qonly"):
                                        nc.vector.tensor_scalar(
                                            spec["out"][:, d, :], acc[:],
                                            spec["bias"][:, d:d + 1], None, OP.add)
                                    else:
                                        ot = sp.tile([128, 512], BF16, tag="kev")
                                        nc.vector.tensor_scalar(
                                            ot[:], acc[:],
                                            spec["bias"][:, d:d + 1], None, OP.add)
                                        nc.sync.dma_start(
                                            spec["out"][d * 128:(d + 1) * 128,
                                                        base + ch * 512:
                                                        base + (ch + 1) * 512],
                                            ot[:])
                    else:  # 'v'
                        for dvc in range(2):
                            bbp = pp.tile([128, 512], F32, tag="mm")
                            nc.tensor.matmul(
                                bbp[:], t_ones_r[:],
                                spec["bias"][:, dvc * 512:(dvc + 1) * 512],
                                start=True, stop=True)
                            bb = sp.tile([128, 512], F32, tag="vbb", bufs=2)
                            nc.vector.tensor_copy(bb[:], bbp[:])
                            strips = []
                            for c in range(CT):
                                t = wp.tile([128, 512], BF16, tag="wstr", name="wstr")
                                nc.scalar.dma_start(
                                    t[:], wd[c * 128:(c + 1) * 128,
                                             dvc * 512:(dvc + 1) * 512])
                                strips.append(t)
                            for tt in range(8):
                                acc = pp.tile([128, 512], F32, tag="mm")
                                for c in range(CT):
                                    nc.tensor.matmul(
                                        acc[:], h[:, c, tt * 128:(tt + 1) * 128],
                                        strips[c][:, :], start=(c == 0),
                                        stop=(c == CT - 1))
                                ot = sp.tile([128, 512], BF16, tag="vev")
                                nc.vector.tensor_add(ot[:], acc[:], bb[:])
                                nc.sync.dma_start(
                                    spec["out"][base + tt * 128:base + (tt + 1) * 128,
                                                dvc * 512:(dvc + 1) * 512],
                                    ot[:])

            def attention(q, k_scr, v_scr, mask_tile, use_maskc, O, sp, pp):
                """q bf16 [128, CT, 512]; K/V streamed bf16 from DRAM scratch.
                Writes O bf16 [64, H, 512] (softmax-normalized per head)."""
                kpair = None
                for h in range(H):
                    dt, pr = h // 2, 64 * (h % 2)
                    if pr == 0:
                        kpair = sp.tile([128, S], BF16, tag="kstr", bufs=2)
                        nc.gpsimd.dma_start(kpair[:],
                                            k_scr[dt * 128:(dt + 1) * 128, :])
                    va = sp.tile([128, TT, 65], BF16, tag="va", bufs=2)
                    nc.gpsimd.dma_start(
                        va[:, :, 0:64],
                        v_scr[:, h * 64:(h + 1) * 64].rearrange(
                            "(t p) d -> p t d", p=128))
                    nc.vector.memset(va[:, :, 64:65], 1.0)
                    grps = []
                    for g in range(TT // NG):
                        sps = pp.tile([128, NG, 512], F32, tag="sc", bufs=1)
                        for j in range(NG):
                            kt = g * NG + j
                            nc.tensor.matmul(
                                sps[:, j, :],
                                kpair[pr:pr + 64, kt * 128:(kt + 1) * 128],
                                q[pr:pr + 64, dt, :],
                                start=True, stop=True)
                        att = sp.tile([128, NG, 512], BF16, tag="att")
                        nc.scalar.activation(att[:], sps[:], AF.Exp)
                        if mask_tile is not None:
                            nc.vector.tensor_mul(att[:], att[:],
                                                 mask_tile[:, g * NG:(g + 1) * NG, :])
                        if use_maskc:
                            for j in range(NG):
                                kt = g * NG + j
                                nc.vector.tensor_scalar(
                                    att[:, j, :], att[:, j, :],
                                    t_maskc[:, kt:kt + 1], None, OP.mult)
                        grps.append(att)
                    avp = pp.tile([65, 512], F32, tag="av")
                    for kt in range(TT):
                        nc.tensor.matmul(avp[:], va[:, kt, :],
                                         grps[kt // NG][:, kt % NG, :],
                                         start=(kt == 0), stop=(kt == TT - 1))
                    rr = sp.tile([65, 512], F32, tag="rr")
                    nc.vector.reciprocal(rr[64:65, :], avp[64:65, :])
                    rbp = pp.tile([64, 512], F32, tag="rb")
                    nc.tensor.matmul(rbp[:], t_ones_sq[64:65, 0:64], rr[64:65, :],
                                     start=True, stop=True)
                    rb = sp.tile([64, 512], F32, tag="rbs")
                    nc.vector.tensor_copy(rb[:], rbp[:])
                    nc.vector.tensor_mul(O[:, h, :], avp[0:64, :], rb[:])

            def out_proj(O, wo_dram, bias_tile, resid_dram, out_dram, sp, pp, wp):
                """out = wo.T @ O + bias_col + resid (fp32), -> out_dram.

                wo strips are loaded per head at partition base 0 so the lhsT
                base matches the O rhs base (matmul requires equal bases)."""
                for oh in range(2):
                    strips = []
                    for h in range(H):
                        t = wp.tile([64, 512], BF16, tag="wstr", name="wstr", bufs=16)
                        nc.scalar.dma_start(t[:], wo_dram[h * 64:(h + 1) * 64,
                                                          oh * 512:(oh + 1) * 512])
                        strips.append(t)
                    for oq in range(4):
                        o = oh * 4 + oq
                        acc = pp.tile([128, 512], F32, tag="mm")
                        for h in range(H):
                            nc.tensor.matmul(
                                acc[:],
                                strips[h][:, oq * 128:(oq + 1) * 128],
                                O[:, h, :], start=(h == 0), stop=(h == H - 1))
                        res = sp.tile([128, 512], F32, tag="res")
                        nc.gpsimd.dma_start(res[:],
                                            resid_dram[o * 128:(o + 1) * 128, 0:512])
                        ot = sp.tile([128, 512], F32, tag="xout")
                        nc.vector.scalar_tensor_tensor(ot[:], acc[:],
                                                       bias_tile[:, o:o + 1],
                                                       res[:], OP.add, OP.add)
                        nc.sync.dma_start(out_dram[o * 128:(o + 1) * 128, :], ot[:])

            def attn_block(src_dram, normalize_src, q_w, q_b, k_w, k_b, v_w, v_br,
                           o_w, o_b, k_scr, v_scr, mask_tile_src, use_maskc,
                           resid_dram, out_dram, q_src_dram, kv_done=False,
                           overlap_emit=None):
                """One full attention block. src_dram: K/V source (fp32 xT for
                SA, bf16 encT for CA). q_src_dram: fp32 LN source for Q when
                not normalize_src (CA: x1_scr)."""
                with tc.tile_pool(name="qkeep", bufs=1) as qkeep:
                    q = qkeep.tile([128, CT, 512], BF16, tag="q")
                    with tc.tile_pool(name="rows", bufs=1) as rows_pool:
                        if normalize_src:
                            mean, rstd = ln_stats(src_dram, S, rows_pool)
                        else:
                            mean, rstd = ln_stats(q_src_dram, CH, rows_pool)
                        with (
                            tc.tile_pool(name="prep", bufs=2) as sp,
                            tc.tile_pool(name="wstr", bufs=16) as wp,
                            tc.tile_pool(name="prepp", bufs=2, space="PSUM") as pp,
                        ):
                            with tc.tile_pool(name="hbuf", bufs=1) as hp:
                                if normalize_src:
                                    # SA: h = LN1(x) bf16, by halves; Q from half 0
                                    h = hp.tile([128, CT, 1024], BF16, tag="h")
                                    for half in range(2):
                                        base = half * 1024
                                        for ch2 in range(2):
                                            chg = half * 2 + ch2
                                            sl = slice(ch2 * 512, (ch2 + 1) * 512)
                                            mb = pp.tile([128, 512], F32, tag="mb")
                                            nc.tensor.matmul(mb[:], t_ones_r[:],
                                                             mean[chg][:],
                                                             start=True, stop=True)
                                            rbb = pp.tile([128, 512], F32, tag="rbb")
                                            nc.tensor.matmul(rbb[:], t_ones_r[:],
                                                             rstd[chg][:],
                                                             start=True, stop=True)
                                            for c in range(CT):
                                                xc = sp.tile([128, 512], F32, tag="xs2")
                                                nc.gpsimd.dma_start(
                                                    xc[:],
                                                    src_dram[c * 128:(c + 1) * 128,
                                                             base + ch2 * 512:
                                                             base + (ch2 + 1) * 512])
                                                nc.vector.tensor_sub(h[:, c, sl],
                                                                     xc[:], mb[:])
                                                nc.vector.tensor_mul(h[:, c, sl],
                                                                     h[:, c, sl],
                                                                     rbb[:])
                                        w_list = [
                                            {"kind": "kT", "w": q_w, "bias": q_b,
                                             "out": q, "qonly": True},
                                            {"kind": "kT", "w": k_w, "bias": k_b,
                                             "out": k_scr},
                                            {"kind": "v", "w": v_w, "bias": v_br,
                                             "out": v_scr},
                                        ]
                                        proj_block(h, sp, pp, wp, half=half,
                                                   w_list=w_list)
                                else:
                                    # CA: Q = LN2(x1) proj; then raw encoder K/V
                                    h2 = hp.tile([128, CT, 1024], BF16, tag="h")
                                    mb = pp.tile([128, 512], F32, tag="mb")
                                    nc.tensor.matmul(mb[:], t_ones_r[:], mean[0][:],
                                                     start=True, stop=True)
                                    rbb = pp.tile([128, 512], F32, tag="rbb")
                                    nc.tensor.matmul(rbb[:], t_ones_r[:], rstd[0][:],
                                                     start=True, stop=True)
                                    for c in range(CT):
                                        xc = sp.tile([128, 512], F32, tag="xs2")
                                        nc.gpsimd.dma_start(
                                            xc[:],
                                            q_src_dram[c * 128:(c + 1) * 128, :])
                                        nc.vector.tensor_sub(h2[:, c, 0:512],
                                                             xc[:], mb[:])
                                        nc.vector.tensor_mul(h2[:, c, 0:512],
                                                             h2[:, c, 0:512], rbb[:])
                                    proj_block(h2, sp, pp, wp, half=0,
                                               w_list=[{"kind": "kT", "w": q_w,
                                                        "bias": q_b, "out": q,
                                                        "qonly": True}])
                                    if not kv_done:
                                        for half in range(2):
                                            base = half * 1024
                                            henc = hp.tile([128, CT, 1024], BF16,
                                                           tag="h", name="henc")
                                            for c in range(CT):
                                                nc.gpsimd.dma_start(
                                                    henc[:, c, :],
                                                    src_dram[c * 128:(c + 1) * 128,
                                                             base:base + 1024])
                                            w_list = [
                                                {"kind": "kT", "w": k_w, "bias": k_b,
                                                 "out": k_scr},
                                                {"kind": "v", "w": v_w, "bias": v_br,
                                                 "out": v_scr},
                                            ]
                                            proj_block(henc, sp, pp, wp, half=half,
                                                       w_list=w_list)
                    # attention + out-proj
                    from contextlib import ExitStack
                    with tc.tile_pool(name="attn_o", bufs=1) as op_, ExitStack() as ovs:
                        O = op_.tile([64, H, 512], BF16, tag="O")
                        if overlap_emit is not None:
                            overlap_emit(ovs)
                        with (
                            tc.tile_pool(name="attn", bufs=4) as sp,
                            tc.tile_pool(name="attnp", bufs=1, space="PSUM") as pp,
                        ):
                            if mask_tile_src is not None:
                                with tc.tile_pool(name="maskp", bufs=1) as mp:
                                    mask_tile = mp.tile([128, TT, 512], BF16, tag="m")
                                    nc.sync.dma_start(
                                        mask_tile[:],
                                        mask_tile_src.rearrange("(t p) q -> p t q",
                                                                p=128))
                                    attention(q, k_scr, v_scr, mask_tile, False,
                                              O, sp, pp)
                            else:
                                attention(q, k_scr, v_scr, None, use_maskc,
                                          O, sp, pp)
                        with (
                            tc.tile_pool(name="oproj", bufs=2) as sp,
                            tc.tile_pool(name="wstro", bufs=1) as wp,
                            tc.tile_pool(name="oprojp", bufs=2, space="PSUM") as pp,
                        ):
                            out_proj(O, o_w, o_b, resid_dram, out_dram, sp, pp, wp)

            # CA K/V production is independent of block 1 -- emit it inside
            # the SA-attention scope so its PE work fills the ACT-bound
            # softmax stretch.
            def ca_kv_overlap(stack):
                csp = stack.enter_context(tc.tile_pool(name="cap", bufs=2))
                cwp = stack.enter_context(tc.tile_pool(name="caw", bufs=16))
                chp = stack.enter_context(tc.tile_pool(name="chb", bufs=1))
                cpp = stack.enter_context(
                    tc.tile_pool(name="capp", bufs=2, space="PSUM"))
                for half in range(2):
                    henc = chp.tile([128, CT, 1024], BF16, tag="h", name="henc")
                    for c in range(CT):
                        nc.gpsimd.dma_start(
                            henc[:, c, :],
                            encT[c * 128:(c + 1) * 128,
                                 half * 1024:(half + 1) * 1024])
                    proj_block(henc, csp, cpp, cwp, half=half, w_list=[
                        {"kind": "kT", "w": w_d["ca_wk"], "bias": t_bc["ca_bk"],
                         "out": k_ca_scr},
                        {"kind": "v", "w": w_d["ca_wv"], "bias": t_br["ca_bv"],
                         "out": v_ca_scr}])

            # ================= Block 1: self-attention =================
            attn_block(xT, True, w_d["sa_wq"], t_bc["sa_bq"], w_d["sa_wk"],
                       t_bc["sa_bk"], w_d["sa_wv"], t_br["sa_bv"], w_d["sa_wo"],
                       t_bc["sa_bo"], k_sa_scr, v_sa_scr, maskT_d, False,
                       xT, x1_scr, None, overlap_emit=ca_kv_overlap)

            # ================= Block 2: cross-attention =================
            attn_block(encT, False, w_d["ca_wq"], t_bc["ca_bq"], w_d["ca_wk"],
                       t_bc["ca_bk"], w_d["ca_wv"], t_br["ca_bv"], w_d["ca_wo"],
                       t_bc["ca_bo"], k_ca_scr, v_ca_scr, None, apply_src_mask,
                       x1_scr, x2_scr, x1_scr, kv_done=True)

            # ================= Block 3: FFN =================
            with tc.tile_pool(name="ffrows", bufs=1) as rows_pool:
                mean3, rstd3 = ln_stats(x2_scr, CH, rows_pool)
                with (
                    tc.tile_pool(name="ffsp", bufs=2) as sp,
                    tc.tile_pool(name="ffw", bufs=4) as wp,
                    tc.tile_pool(name="ffbig", bufs=1) as bigp,
                    tc.tile_pool(name="ffpp", bufs=2, space="PSUM") as pp,
                    tc.tile_pool(name="ffacc", bufs=1, space="PSUM") as accp,
                ):
                    h3 = bigp.tile([128, CT, 512], BF16, tag="h3")
                    mb = pp.tile([128, 512], F32, tag="mm")
                    nc.tensor.matmul(mb[:], t_ones_r[:], mean3[0][:],
                                     start=True, stop=True)
                    rbb = pp.tile([128, 512], F32, tag="mm")
                    nc.tensor.matmul(rbb[:], t_ones_r[:], rstd3[0][:],
                                     start=True, stop=True)
                    for c in range(CT):
                        xc = sp.tile([128, 512], F32, tag="xs3")
                        nc.gpsimd.dma_start(xc[:], x2_scr[c * 128:(c + 1) * 128, :])
                        nc.vector.tensor_sub(h3[:, c, :], xc[:], mb[:])
                        nc.vector.tensor_mul(h3[:, c, :], h3[:, c, :], rbb[:])
                    g = bigp.tile([128, FT, 512], BF16, tag="g")
                    for fh in range(2):
                        strips = []
                        for c in range(CT):
                            t = wp.tile([128, 2048], BF16, tag="w1s", name="w1s",
                                        bufs=8)
                            nc.scalar.dma_start(
                                t[:], w_d["ff_w1"][c * 128:(c + 1) * 128,
                                                   fh * 2048:(fh + 1) * 2048])
                            strips.append(t)
                        for fq in range(16):
                            f = fh * 16 + fq
                            acc = pp.tile([128, 512], F32, tag="mm")
                            for c in range(CT):
                                nc.tensor.matmul(
                                    acc[:], strips[c][:, fq * 128:(fq + 1) * 128],
                                    h3[:, c, :], start=(c == 0), stop=(c == CT - 1))
                            # relu(x + b1) on DVE: (acc + bias) max 0 -> bf16
                            nc.vector.tensor_scalar(g[:, f, :], acc[:],
                                                    t_fb1[:, f:f + 1], 0.0,
                                                    OP.add, OP.max)
                    for oh in range(2):
                        accs = [accp.tile([128, 512], F32, tag=f"acc{i}",
                                          name=f"acc{i}") for i in range(4)]
                        for f in range(FT):
                            w2s = wp.tile([128, 512], BF16, tag="w2s", name="w2s",
                                          bufs=8)
                            nc.scalar.dma_start(
                                w2s[:], w_d["ff_w2"][f * 128:(f + 1) * 128,
                                                     oh * 512:(oh + 1) * 512])
                            for oq in range(4):
                                nc.tensor.matmul(accs[oq][:],
                                                 w2s[:, oq * 128:(oq + 1) * 128],
                                                 g[:, f, :],
                                                 start=(f == 0), stop=(f == FT - 1))
                        for oq in range(4):
                            o = oh * 4 + oq
                            res = sp.tile([128, 512], F32, tag="res3")
                            nc.gpsimd.dma_start(res[:],
                                                x2_scr[o * 128:(o + 1) * 128, :])
                            ot = sp.tile([128, 512], F32, tag="yev")
                            nc.vector.scalar_tensor_tensor(
                                ot[:], accs[oq][:], t_bc["ff_b2"][:, o:o + 1],
                                res[:], OP.add, OP.add)
                            nc.sync.dma_start(yT[o * 128:(o + 1) * 128, :], ot[:])

    nc.compile()
    return nc


def _prep_host(inputs):
    """Host-side folds and per-core data prep."""
    import ml_dtypes
    BF = ml_dtypes.bfloat16
    f32 = lambda a: np.ascontiguousarray(np.asarray(a, np.float32))
    bf = lambda a: np.ascontiguousarray(np.asarray(a, np.float32).astype(BF))
    x = f32(inputs["x"])
    enc = f32(inputs["encoder_output"])
    tgt = np.asarray(inputs["tgt_mask"])[0, 0].astype(np.float32)     # [S, S]
    src = np.asarray(inputs["src_mask"])[0, 0, 0].astype(np.float32)  # [S]
    g1, b1 = f32(inputs["n1_g"]), f32(inputs["n1_b"])
    g2, b2 = f32(inputs["n2_g"]), f32(inputs["n2_b"])
    g3, b3 = f32(inputs["n3_g"]), f32(inputs["n3_b"])
    scale = np.float32(1.0 / np.sqrt(DK))

    w = {}
    w["sa_wq"] = bf((g1[:, None] * f32(inputs["sa_wq"])) * scale)
    sa_bq = (b1 @ f32(inputs["sa_wq"]) + f32(inputs["sa_bq"])) * scale
    w["sa_wk"] = bf(g1[:, None] * f32(inputs["sa_wk"]))
    sa_bk = b1 @ f32(inputs["sa_wk"]) + f32(inputs["sa_bk"])
    w["sa_wv"] = bf(g1[:, None] * f32(inputs["sa_wv"]))
    sa_bv = b1 @ f32(inputs["sa_wv"]) + f32(inputs["sa_bv"])
    w["sa_wo"] = bf(inputs["sa_wo"])
    sa_bo = f32(inputs["sa_bo"])
    w["ca_wq"] = bf((g2[:, None] * f32(inputs["ca_wq"])) * scale)
    ca_bq = (b2 @ f32(inputs["ca_wq"]) + f32(inputs["ca_bq"])) * scale
    w["ca_wk"] = bf(inputs["ca_wk"])
    ca_bk = f32(inputs["ca_bk"])
    w["ca_wv"] = bf(inputs["ca_wv"])
    ca_bv = f32(inputs["ca_bv"])
    w["ca_wo"] = bf(inputs["ca_wo"])
    ca_bo = f32(inputs["ca_bo"])
    w["ff_w1"] = bf(g3[:, None] * f32(inputs["ff_w1"]))
    ff_b1 = b3 @ f32(inputs["ff_w1"]) + f32(inputs["ff_b1"])
    w["ff_w2"] = bf(inputs["ff_w2"])
    ff_b2 = f32(inputs["ff_b2"])

    col = lambda b: np.ascontiguousarray(np.asarray(b, np.float32).reshape(-1, 128).T)
    row = lambda b: np.ascontiguousarray(np.asarray(b, np.float32).reshape(1, -1))
    shared = dict(w)
    shared["sa_bq"] = col(sa_bq)
    shared["sa_bk"] = col(sa_bk)
    shared["sa_bo"] = col(sa_bo)
    shared["ca_bq"] = col(ca_bq)
    shared["ca_bk"] = col(ca_bk)
    shared["ca_bo"] = col(ca_bo)
    shared["ff_b2"] = col(ff_b2)
    shared["sa_bv"] = row(sa_bv)
    shared["ca_bv"] = row(ca_bv)
    shared["ff_b1"] = col(ff_b1)
    shared["ones_r"] = np.ones((1, 128), np.float32)
    shared["ones_c"] = np.ones((128, 1), np.float32)

    apply_src_mask = not bool(np.all(src == 1.0))
    if apply_src_mask:
        shared["maskc"] = np.ascontiguousarray(src.reshape(S, 1).astype(BF))

    in_maps = []
    for core in range(8):
        b, c = core // 4, core % 4
        q0 = c * CH
        perm = np.r_[q0:q0 + CH, 0:q0, q0 + CH:S]
        m = dict(shared)
        m["xT"] = np.ascontiguousarray(x[b].T[:, perm])
        m["encT"] = np.ascontiguousarray(enc[b].T.astype(BF))
        m["maskT"] = np.ascontiguousarray(tgt[q0:q0 + CH, :].T[perm, :].astype(BF))
        in_maps.append(m)
    return in_maps, apply_src_mask


def kernel(**inputs):
    from concourse.bass_utils import run_bass_kernel_spmd

    in_maps, apply_src_mask = _prep_host(inputs)
    key = apply_src_mask
    if key not in _CACHE:
        _CACHE[key] = _build(apply_src_mask)
    nc = _CACHE[key]
    res = run_bass_kernel_spmd(nc, in_maps, core_ids=list(range(8)))
    out = np.empty((2, S, D), np.float32)
    for core in range(8):
        b, c = core // 4, core % 4
        out[b, c * CH:(c + 1) * CH, :] = res.results[core]["yT"].T
    return out


# revision 17
# speedup vs baseline: 2.5915x; 1.0539x over previous
"""Trainium2 Bass kernel for a pre-norm transformer decoder layer.

kernel(**inputs) takes the full unsharded inputs of reference.setup_inputs()
and returns the full [2, 2048, 1024] fp32 output.

Sharding: 8 NeuronCores, token-parallel, zero collectives. Core i handles
batch b = i // 4 and query chunk c = i % 4 (512 tokens). Each core computes
the full-batch K/V projections it needs locally (SA K/V from LN1(x) of its
batch; CA K/V from the raw encoder output). The token axis is rolled per
core so its own query chunk sits at positions [0:512) -- one SPMD program,
per-core data only.

Numerics: matmul OPERANDS are bf16 (fp32 runs as two PE passes -- half
throughput); accumulation is always fp32 in PSUM. The LayerNorm statistics,
softmax denominators/reciprocals, biases, and the entire residual stream
stay fp32, so rounding error does not compound across blocks.

Layout is feature-major (xT [D, T]): weights load as lhsT with the
contraction on partitions; no activation transposes anywhere. LayerNorm
stats use ones-column matmuls (partition reductions on PE), softmax runs
without max-subtraction (scores are O(1) here by construction), the mask is
multiplicative 0/1 applied after exp (exact for any mask content), and the
softmax denominator falls out of an appended ones-column in V. gamma/beta
and the attention scale are folded into the weights on the host. K^T and V
are spilled to DRAM scratch (bf16) and streamed back per head.
"""

import sys
sys.path.insert(0, "/opt/trn_rl_repo")

import numpy as np

D = 1024
H = 16
DK = 64
DFF = 4096
S = 2048
CH = 512
EPS = 1e-6
CT = D // 128    # 8 feature tiles
TT = S // 128    # 16 token tiles
FT = DFF // 128  # 32 ff tiles
NG = 2           # score k-tiles per exp/mask group

_CACHE = {}


def _build(apply_src_mask: bool):
    import concourse.bacc as bacc
    import concourse.tile as tile
    from concourse import mybir

    F32 = mybir.dt.float32
    BF16 = mybir.dt.bfloat16
    AF = mybir.ActivationFunctionType
    OP = mybir.AluOpType

    nc = bacc.Bacc("TRN2", target_bir_lowering=False, debug=False)

    xT = nc.dram_tensor("xT", [D, S], F32, kind="ExternalInput")
    encT = nc.dram_tensor("encT", [D, S], BF16, kind="ExternalInput")
    maskT_d = nc.dram_tensor("maskT", [S, CH], BF16, kind="ExternalInput")
    maskc_d = None
    if apply_src_mask:
        maskc_d = nc.dram_tensor("maskc", [S, 1], BF16, kind="ExternalInput")
    w_d = {}
    for nm in ("sa_wq", "sa_wk", "sa_wv", "sa_wo", "ca_wq", "ca_wk", "ca_wv", "ca_wo"):
        w_d[nm] = nc.dram_tensor(nm, [D, D], BF16, kind="ExternalInput")
    w_d["ff_w1"] = nc.dram_tensor("ff_w1", [D, DFF], BF16, kind="ExternalInput")
    w_d["ff_w2"] = nc.dram_tensor("ff_w2", [DFF, D], BF16, kind="ExternalInput")
    bc_d = {}  # bias columns [128, CT] fp32
    for nm in ("sa_bq", "sa_bk", "sa_bo", "ca_bq", "ca_bk", "ca_bo", "ff_b2"):
        bc_d[nm] = nc.dram_tensor(nm, [128, CT], F32, kind="ExternalInput")
    br_d = {}  # bias rows [1, D] fp32 (rank-1 adds on token-major outputs)
    for nm in ("sa_bv", "ca_bv"):
        br_d[nm] = nc.dram_tensor(nm, [1, D], F32, kind="ExternalInput")
    fb1_d = nc.dram_tensor("ff_b1", [128, FT], F32, kind="ExternalInput")
    ones_r_d = nc.dram_tensor("ones_r", [1, 128], F32, kind="ExternalInput")
    ones_c_d = nc.dram_tensor("ones_c", [128, 1], F32, kind="ExternalInput")
    yT = nc.dram_tensor("yT", [D, CH], F32, kind="ExternalOutput")

    with tile.TileContext(nc) as tc:
        with (
            tc.tile_pool(name="const", bufs=1) as constp,
            tc.tile_pool(name="dram", bufs=1, space="DRAM") as dram,
        ):
            t_ones_r = constp.tile([1, 128], F32, tag="ones_r")
            nc.sync.dma_start(t_ones_r[:], ones_r_d[:])
            t_ones_sq = constp.tile([128, 128], F32, tag="ones_sq")
            nc.vector.memset(t_ones_sq[:], 1.0)
            t_ones_c = constp.tile([128, 1], F32, tag="ones_c")
            nc.sync.dma_start(t_ones_c[:], ones_c_d[:])
            t_bc = {}
            for nm, hnd in bc_d.items():
                t_bc[nm] = constp.tile([128, CT], F32, tag=f"b_{nm}", name=f"b_{nm}")
                nc.sync.dma_start(t_bc[nm][:], hnd[:])
            t_br = {}
            for nm, hnd in br_d.items():
                t_br[nm] = constp.tile([1, D], F32, tag=f"b_{nm}", name=f"b_{nm}")
                nc.sync.dma_start(t_br[nm][:], hnd[:])
            t_fb1 = constp.tile([128, FT], F32, tag="b_ff_b1")
            nc.sync.dma_start(t_fb1[:], fb1_d[:])
            t_maskc = None
            if apply_src_mask:
                t_maskc = constp.tile([128, TT], BF16, tag="maskc")
                nc.sync.dma_start(t_maskc[:],
                                  maskc_d.rearrange("(t p) o -> p (t o)", p=128))

            # DRAM scratch (K/V in bf16; residual stream fp32)
            k_sa_scr = dram.tile([D, S], BF16, tag="k_sa")
            v_sa_scr = dram.tile([S, D], BF16, tag="v_sa")
            k_ca_scr = dram.tile([D, S], BF16, tag="k_ca")
            v_ca_scr = dram.tile([S, D], BF16, tag="v_ca")
            x1_scr = dram.tile([D, CH], F32, tag="x1")
            x2_scr = dram.tile([D, CH], F32, tag="x2")

            # ------------------------------------------------------------
            # helpers
            # ------------------------------------------------------------

            def ln_stats(src_dram, ntok, rows_pool):
                """Feature-major fp32 DRAM src [D, ntok] -> (mean, rstd) lists
                of [1, 512] fp32 SBUF tiles in rows_pool."""
                nch = ntok // 512
                mean = [rows_pool.tile([1, 512], F32, tag=f"mean{i}", name=f"mean{i}")
                        for i in range(nch)]
                rstd = [rows_pool.tile([1, 512], F32, tag=f"rstd{i}", name=f"rstd{i}")
                        for i in range(nch)]
                with (
                    tc.tile_pool(name="lns", bufs=2) as sp,
                    tc.tile_pool(name="lnp", bufs=1, space="PSUM") as pp,
                ):
                    s1c = [pp.tile([1, 512], F32, tag=f"s1_{i}", name=f"s1_{i}")
                           for i in range(nch)]
                    s2c = [pp.tile([1, 512], F32, tag=f"s2_{i}", name=f"s2_{i}")
                           for i in range(nch)]
                    for c in range(CT):
                        xc = sp.tile([128, ntok], F32, tag="xs")
                        nc.gpsimd.dma_start(xc[:], src_dram[c * 128:(c + 1) * 128, :])
                        xsq = sp.tile([128, ntok], F32, tag="sq")
                        nc.scalar.activation(xsq[:], xc[:], AF.Square)
                        for ch in range(nch):
                            sl = slice(ch * 512, (ch + 1) * 512)
                            nc.tensor.matmul(s1c[ch][:], t_ones_c[:], xc[:, sl],
                                             start=(c == 0), stop=(c == CT - 1))
                            nc.tensor.matmul(s2c[ch][:], t_ones_c[:], xsq[:, sl],
                                             start=(c == 0), stop=(c == CT - 1))
                    for ch in range(nch):
                        t1 = sp.tile([1, 512], F32, tag="t1")
                        t2 = sp.tile([1, 512], F32, tag="t2")
                        nc.vector.tensor_scalar_mul(mean[ch][:], s1c[ch][:], 1.0 / D)
                        nc.vector.tensor_mul(t1[:], s1c[ch][:], mean[ch][:])
                        nc.vector.tensor_sub(t1[:], s2c[ch][:], t1[:])
                        nc.vector.tensor_scalar_mul(t1[:], t1[:], 1.0 / (D - 1))
                        nc.scalar.activation(t2[:], t1[:], AF.Sqrt)
                        nc.vector.tensor_scalar_add(t2[:], t2[:], EPS)
                        nc.vector.reciprocal(rstd[ch][:], t2[:])
                return mean, rstd

            def proj_block(h, sp, pp, wp, *, half, w_list):
                """Projections for one token half. h: bf16 [128, CT, 1024].

                spec kind 'kT': out feature-major (rhs = h, lhsT = weight),
                evict + bias_col -> bf16 scr [D, S] (or SBUF q when qonly).
                spec kind 'v': out token-major (lhsT = h, rhs = weight),
                rank-1 fp32 bias row, evict -> bf16 scr [S, D]."""
                base = half * 1024
                for spec in w_list:
                    if spec.get("qonly") and half != 0:
                        continue
                    wd = spec["w"]
                    if spec["kind"] == "kT":
                        nch = 1 if spec.get("qonly") else 2
                        for dh in range(2):
                            strips = []
                            for c in range(CT):
                                t = wp.tile([128, 512], BF16, tag="wstr", name="wstr")
                                nc.scalar.dma_start(
                                    t[:], wd[c * 128:(c + 1) * 128,
                                             dh * 512:(dh + 1) * 512])
                                strips.append(t)
                            for dq in range(4):
                                d = dh * 4 + dq
                                for ch in range(nch):
                                    sl = slice(ch * 512, (ch + 1) * 512)
                                    acc = pp.tile([128, 512], F32, tag="mm")
                                    for c in range(CT):
                                        nc.tensor.matmul(
                                            acc[:],
                                            strips[c][:, dq * 128:(dq + 1) * 128],
                                            h[:, c, sl],
                                            start=(c == 0), stop=(c == CT - 1))
                                    if spec.get("qonly"):
                                        nc.vector.tensor_scalar(
                                            spec["out"][:, d, :], acc[:],
                                            spec["bias"][:, d:d + 1], None, OP.add)
                                    else:
                                        ot = sp.tile([128, 512], BF16, tag="kev")
                                        nc.vector.tensor_scalar(
                                            ot[:], acc[:],
                                            spec["bias"][:, d:d + 1], None, OP.add)
                                        nc.sync.dma_start(
                                            spec["out"][d * 128:(d + 1) * 128,
                                                        base + ch * 512:
                                                        base + (ch + 1) * 512],
                                            ot[:])
                    else:  # 'v'
                        for dvc in range(2):
                            bbp = pp.tile([128, 512], F32, tag="mm")
                            nc.tensor.matmul(
                                bbp[:], t_ones_r[:],
                                spec["bias"][:, dvc * 512:(dvc + 1) * 512],
                                start=True, stop=True)
                            bb = sp.tile([128, 512], F32, tag="vbb", bufs=2)
                            nc.vector.tensor_copy(bb[:], bbp[:])
                            strips = []
                            for c in range(CT):
                                t = wp.tile([128, 512], BF16, tag="wstr", name="wstr")
                                nc.scalar.dma_start(
                                    t[:], wd[c * 128:(c + 1) * 128,
                                             dvc * 512:(dvc + 1) * 512])
                                strips.append(t)
                            for tt in range(8):
                                acc = pp.tile([128, 512], F32, tag="mm")
                                for c in range(CT):
                                    nc.tensor.matmul(
                                        acc[:], h[:, c, tt * 128:(tt + 1) * 128],
                                        strips[c][:, :], start=(c == 0),
                                        stop=(c == CT - 1))
                                ot = sp.tile([128, 512], BF16, tag="vev")
                                nc.vector.tensor_add(ot[:], acc[:], bb[:])
                                nc.sync.dma_start(
                                    spec["out"][base + tt * 128:base + (tt + 1) * 128,
                                                dvc * 512:(dvc + 1) * 512],
                                    ot[:])

            def attention(q, k_scr, v_scr, mask_tile, use_maskc, O, sp, pp):
                """q bf16 [128, CT, 512]; K/V streamed bf16 from DRAM scratch.
                Writes O bf16 [64, H, 512] (softmax-normalized per head)."""
                kpair = None
                for h in range(H):
                    dt, pr = h // 2, 64 * (h % 2)
                    if pr == 0:
                        kpair = sp.tile([128, S], BF16, tag="kstr", bufs=2)
                        nc.gpsimd.dma_start(kpair[:],
                                            k_scr[dt * 128:(dt + 1) * 128, :])
                    va = sp.tile([128, TT, 65], BF16, tag="va", bufs=2)
                    nc.gpsimd.dma_start(
                        va[:, :, 0:64],
                        v_scr[:, h * 64:(h + 1) * 64].rearrange(
                            "(t p) d -> p t d", p=128))
                    nc.vector.memset(va[:, :, 64:65], 1.0)
                    grps = []
                    for g in range(TT // NG):
                        sps = pp.tile([128, NG, 512], F32, tag="sc", bufs=2)
                        for j in range(NG):
                            kt = g * NG + j
                            nc.tensor.matmul(
                                sps[:, j, :],
                                kpair[pr:pr + 64, kt * 128:(kt + 1) * 128],
                                q[pr:pr + 64, dt, :],
                                start=True, stop=True)
                        att = sp.tile([128, NG, 512], BF16, tag="att", bufs=12)
                        nc.scalar.activation(att[:], sps[:], AF.Exp)
                        if mask_tile is not None:
                            nc.vector.tensor_mul(att[:], att[:],
                                                 mask_tile[:, g * NG:(g + 1) * NG, :])
                        if use_maskc:
                            for j in range(NG):
                                kt = g * NG + j
                                nc.vector.tensor_scalar(
                                    att[:, j, :], att[:, j, :],
                                    t_maskc[:, kt:kt + 1], None, OP.mult)
                        grps.append(att)
                    avp = pp.tile([65, 512], F32, tag="av")
                    for kt in range(TT):
                        nc.tensor.matmul(avp[:], va[:, kt, :],
                                         grps[kt // NG][:, kt % NG, :],
                                         start=(kt == 0), stop=(kt == TT - 1))
                    rr = sp.tile([65, 512], F32, tag="rr")
                    nc.vector.reciprocal(rr[64:65, :], avp[64:65, :])
                    rbp = pp.tile([64, 512], F32, tag="rb")
                    nc.tensor.matmul(rbp[:], t_ones_sq[64:65, 0:64], rr[64:65, :],
                                     start=True, stop=True)
                    rb = sp.tile([64, 512], F32, tag="rbs")
                    nc.vector.tensor_copy(rb[:], rbp[:])
                    nc.vector.tensor_mul(O[:, h, :], avp[0:64, :], rb[:])

            def out_proj(O, wo_dram, bias_tile, resid_dram, out_dram, sp, pp, wp):
                """out = wo.T @ O + bias_col + resid (fp32), -> out_dram.

                wo strips are loaded per head at partition base 0 so the lhsT
                base matches the O rhs base (matmul requires equal bases)."""
                for oh in range(2):
                    strips = []
                    for h in range(H):
                        t = wp.tile([64, 512], BF16, tag="wstr", name="wstr", bufs=16)
                        nc.scalar.dma_start(t[:], wo_dram[h * 64:(h + 1) * 64,
                                                          oh * 512:(oh + 1) * 512])
                        strips.append(t)
                    for oq in range(4):
                        o = oh * 4 + oq
                        acc = pp.tile([128, 512], F32, tag="mm")
                        for h in range(H):
                            nc.tensor.matmul(
                                acc[:],
                                strips[h][:, oq * 128:(oq + 1) * 128],
                                O[:, h, :], start=(h == 0), stop=(h == H - 1))
                        res = sp.tile([128, 512], F32, tag="res")
                        nc.gpsimd.dma_start(res[:],
                                            resid_dram[o * 128:(o + 1) * 128, 0:512])
                        ot = sp.tile([128, 512], F32, tag="xout")
                        nc.vector.scalar_tensor_tensor(ot[:], acc[:],
                                                       bias_tile[:, o:o + 1],
                                                       res[:], OP.add, OP.add)
                        nc.sync.dma_start(out_dram[o * 128:(o + 1) * 128, :], ot[:])

            def attn_block(src_dram, normalize_src, q_w, q_b, k_w, k_b, v_w, v_br,
                           o_w, o_b, k_scr, v_scr, mask_tile_src, use_maskc,
                           resid_dram, out_dram, q_src_dram, kv_done=False,
                           overlap_emit=None):
                """One full attention block. src_dram: K/V source (fp32 xT for
                SA, bf16 encT for CA). q_src_dram: fp32 LN source for Q when
                not normalize_src (CA: x1_scr)."""
                with tc.tile_pool(name="qkeep", bufs=1) as qkeep:
                    q = qkeep.tile([128, CT, 512], BF16, tag="q")
                    with tc.tile_pool(name="rows", bufs=1) as rows_pool:
                        if normalize_src:
                            mean, rstd = ln_stats(src_dram, S, rows_pool)
                        else:
                            mean, rstd = ln_stats(q_src_dram, CH, rows_pool)
                        with (
                            tc.tile_pool(name="prep", bufs=2) as sp,
                            tc.tile_pool(name="wstr", bufs=16) as wp,
                            tc.tile_pool(name="prepp", bufs=2, space="PSUM") as pp,
                        ):
                            with tc.tile_pool(name="hbuf", bufs=1) as hp:
                                if normalize_src:
                                    # SA: h = LN1(x) bf16, by halves; Q from half 0
                                    h = hp.tile([128, CT, 1024], BF16, tag="h")
                                    for half in range(2):
                                        base = half * 1024
                                        for ch2 in range(2):
                                            chg = half * 2 + ch2
                                            sl = slice(ch2 * 512, (ch2 + 1) * 512)
                                            mb = pp.tile([128, 512], F32, tag="mb")
                                            nc.tensor.matmul(mb[:], t_ones_r[:],
                                                             mean[chg][:],
                                                             start=True, stop=True)
                                            rbb = pp.tile([128, 512], F32, tag="rbb")
                                            nc.tensor.matmul(rbb[:], t_ones_r[:],
                                                             rstd[chg][:],
                                                             start=True, stop=True)
                                            for c in range(CT):
                                                xc = sp.tile([128, 512], F32, tag="xs2")
                                                nc.gpsimd.dma_start(
                                                    xc[:],
                                                    src_dram[c * 128:(c + 1) * 128,
                                                             base + ch2 * 512:
                                                             base + (ch2 + 1) * 512])
                                                nc.vector.tensor_sub(h[:, c, sl],
                                                                     xc[:], mb[:])
                                                nc.vector.tensor_mul(h[:, c, sl],
                                                                     h[:, c, sl],
                                                                     rbb[:])
                                        w_list = [
                                            {"kind": "kT", "w": q_w, "bias": q_b,
                                             "out": q, "qonly": True},
                                            {"kind": "kT", "w": k_w, "bias": k_b,
                                             "out": k_scr},
                                            {"kind": "v", "w": v_w, "bias": v_br,
                                             "out": v_scr},
                                        ]
                                        proj_block(h, sp, pp, wp, half=half,
                                                   w_list=w_list)
                                else:
                                    # CA: Q = LN2(x1) proj; then raw encoder K/V
                                    h2 = hp.tile([128, CT, 1024], BF16, tag="h")
                                    mb = pp.tile([128, 512], F32, tag="mb")
                                    nc.tensor.matmul(mb[:], t_ones_r[:], mean[0][:],
                                                     start=True, stop=True)
                                    rbb = pp.tile([128, 512], F32, tag="rbb")
                                    nc.tensor.matmul(rbb[:], t_ones_r[:], rstd[0][:],
                                                     start=True, stop=True)
                                    for c in range(CT):
                                        xc = sp.tile([128, 512], F32, tag="xs2")
                                        nc.gpsimd.dma_start(
                                            xc[:],
                                            q_src_dram[c * 128:(c + 1) * 128, :])
                                        nc.vector.tensor_sub(h2[:, c, 0:512],
                                                             xc[:], mb[:])
                                        nc.vector.tensor_mul(h2[:, c, 0:512],
                                                             h2[:, c, 0:512], rbb[:])
                                    proj_block(h2, sp, pp, wp, half=0,
                                               w_list=[{"kind": "kT", "w": q_w,
                                                        "bias": q_b, "out": q,
                                                        "qonly": True}])
                                    if not kv_done:
                                        for half in range(2):
                                            base = half * 1024
                                            henc = hp.tile([128, CT, 1024], BF16,
                                                           tag="h", name="henc")
                                            for c in range(CT):
                                                nc.gpsimd.dma_start(
                                                    henc[:, c, :],
                                                    src_dram[c * 128:(c + 1) * 128,
                                                             base:base + 1024])
                                            w_list = [
                                                {"kind": "kT", "w": k_w, "bias": k_b,
                                                 "out": k_scr},
                                                {"kind": "v", "w": v_w, "bias": v_br,
                                                 "out": v_scr},
                                            ]
                                            proj_block(henc, sp, pp, wp, half=half,
                                                       w_list=w_list)
                    # attention + out-proj
                    from contextlib import ExitStack
                    with tc.tile_pool(name="attn_o", bufs=1) as op_, ExitStack() as ovs:
                        O = op_.tile([64, H, 512], BF16, tag="O")
                        if overlap_emit is not None:
                            overlap_emit(ovs)
                        with (
                            tc.tile_pool(name="attn", bufs=4) as sp,
                            tc.tile_pool(name="attnp", bufs=1, space="PSUM") as pp,
                        ):
                            if mask_tile_src is not None:
                                with tc.tile_pool(name="maskp", bufs=1) as mp:
                                    mask_tile = mp.tile([128, TT, 512], BF16, tag="m")
                                    nc.sync.dma_start(
                                        mask_tile[:],
                                        mask_tile_src.rearrange("(t p) q -> p t q",
                                                                p=128))
                                    attention(q, k_scr, v_scr, mask_tile, False,
                                              O, sp, pp)
                            else:
                                attention(q, k_scr, v_scr, None, use_maskc,
                                          O, sp, pp)
                        with (
                            tc.tile_pool(name="oproj", bufs=2) as sp,
                            tc.tile_pool(name="wstro", bufs=1) as wp,
                            tc.tile_pool(name="oprojp", bufs=2, space="PSUM") as pp,
                        ):
                            out_proj(O, o_w, o_b, resid_dram, out_dram, sp, pp, wp)

            # CA K/V production is independent of block 1 -- emit it inside
            # the SA-attention scope so its PE work fills the ACT-bound
            # softmax stretch.
            def ca_kv_overlap(stack):
                csp = stack.enter_context(tc.tile_pool(name="cap", bufs=2))
                cwp = stack.enter_context(tc.tile_pool(name="caw", bufs=16))
                chp = stack.enter_context(tc.tile_pool(name="chb", bufs=1))
                cpp = stack.enter_context(
                    tc.tile_pool(name="capp", bufs=2, space="PSUM"))
                for half in range(2):
                    henc = chp.tile([128, CT, 1024], BF16, tag="h", name="henc")
                    for c in range(CT):
                        nc.gpsimd.dma_start(
                            henc[:, c, :],
                            encT[c * 128:(c + 1) * 128,
                                 half * 1024:(half + 1) * 1024])
                    proj_block(henc, csp, cpp, cwp, half=half, w_list=[
                        {"kind": "kT", "w": w_d["ca_wk"], "bias": t_bc["ca_bk"],
                         "out": k_ca_scr},
                        {"kind": "v", "w": w_d["ca_wv"], "bias": t_br["ca_bv"],
                         "out": v_ca_scr}])

            # ================= Block 1: self-attention =================
            attn_block(xT, True, w_d["sa_wq"], t_bc["sa_bq"], w_d["sa_wk"],
                       t_bc["sa_bk"], w_d["sa_wv"], t_br["sa_bv"], w_d["sa_wo"],
                       t_bc["sa_bo"], k_sa_scr, v_sa_scr, maskT_d, False,
                       xT, x1_scr, None, overlap_emit=ca_kv_overlap)

            # ================= Block 2: cross-attention =================
            attn_block(encT, False, w_d["ca_wq"], t_bc["ca_bq"], w_d["ca_wk"],
                       t_bc["ca_bk"], w_d["ca_wv"], t_br["ca_bv"], w_d["ca_wo"],
                       t_bc["ca_bo"], k_ca_scr, v_ca_scr, None, apply_src_mask,
                       x1_scr, x2_scr, x1_scr, kv_done=True)

            # ================= Block 3: FFN =================
            with tc.tile_pool(name="ffrows", bufs=1) as rows_pool:
                mean3, rstd3 = ln_stats(x2_scr, CH, rows_pool)
                with (
                    tc.tile_pool(name="ffsp", bufs=2) as sp,
                    tc.tile_pool(name="ffw", bufs=4) as wp,
                    tc.tile_pool(name="ffbig", bufs=1) as bigp,
                    tc.tile_pool(name="ffpp", bufs=2, space="PSUM") as pp,
                    tc.tile_pool(name="ffacc", bufs=1, space="PSUM") as accp,
                ):
                    h3 = bigp.tile([128, CT, 512], BF16, tag="h3")
                    mb = pp.tile([128, 512], F32, tag="mm")
                    nc.tensor.matmul(mb[:], t_ones_r[:], mean3[0][:],
                                     start=True, stop=True)
                    rbb = pp.tile([128, 512], F32, tag="mm")
                    nc.tensor.matmul(rbb[:], t_ones_r[:], rstd3[0][:],
                                     start=True, stop=True)
                    for c in range(CT):
                        xc = sp.tile([128, 512], F32, tag="xs3")
                        nc.gpsimd.dma_start(xc[:], x2_scr[c * 128:(c + 1) * 128, :])
                        nc.vector.tensor_sub(h3[:, c, :], xc[:], mb[:])
                        nc.vector.tensor_mul(h3[:, c, :], h3[:, c, :], rbb[:])
                    g = bigp.tile([128, FT, 512], BF16, tag="g")
                    for fh in range(2):
                        strips = []
                        for c in range(CT):
                            t = wp.tile([128, 2048], BF16, tag="w1s", name="w1s",
                                        bufs=8)
                            nc.scalar.dma_start(
                                t[:], w_d["ff_w1"][c * 128:(c + 1) * 128,
                                                   fh * 2048:(fh + 1) * 2048])
                            strips.append(t)
                        for fq in range(16):
                            f = fh * 16 + fq
                            acc = pp.tile([128, 512], F32, tag="mm")
                            for c in range(CT):
                                nc.tensor.matmul(
                                    acc[:], strips[c][:, fq * 128:(fq + 1) * 128],
                                    h3[:, c, :], start=(c == 0), stop=(c == CT - 1))
                            # relu(x + b1) on DVE: (acc + bias) max 0 -> bf16
                            nc.vector.tensor_scalar(g[:, f, :], acc[:],
                                                    t_fb1[:, f:f + 1], 0.0,
                                                    OP.add, OP.max)
                    for oh in range(2):
                        accs = [accp.tile([128, 512], F32, tag=f"acc{i}",
                                          name=f"acc{i}") for i in range(4)]
                        for f in range(FT):
                            w2s = wp.tile([128, 512], BF16, tag="w2s", name="w2s",
                                          bufs=8)
                            nc.scalar.dma_start(
                                w2s[:], w_d["ff_w2"][f * 128:(f + 1) * 128,
                                                     oh * 512:(oh + 1) * 512])
                            for oq in range(4):
                                nc.tensor.matmul(accs[oq][:],
                                                 w2s[:, oq * 128:(oq + 1) * 128],
                                                 g[:, f, :],
                                                 start=(f == 0), stop=(f == FT - 1))
                        for oq in range(4):
                            o = oh * 4 + oq
                            res = sp.tile([128, 512], F32, tag="res3")
                            nc.gpsimd.dma_start(res[:],
                                                x2_scr[o * 128:(o + 1) * 128, :])
                            ot = sp.tile([128, 512], F32, tag="yev")
                            nc.vector.scalar_tensor_tensor(
                                ot[:], accs[oq][:], t_bc["ff_b2"][:, o:o + 1],
                                res[:], OP.add, OP.add)
                            nc.sync.dma_start(yT[o * 128:(o + 1) * 128, :], ot[:])

    nc.compile()
    return nc


def _prep_host(inputs):
    """Host-side folds and per-core data prep."""
    import ml_dtypes
    BF = ml_dtypes.bfloat16
    f32 = lambda a: np.ascontiguousarray(np.asarray(a, np.float32))
    bf = lambda a: np.ascontiguousarray(np.asarray(a, np.float32).astype(BF))
    x = f32(inputs["x"])
    enc = f32(inputs["encoder_output"])
    tgt = np.asarray(inputs["tgt_mask"])[0, 0].astype(np.float32)     # [S, S]
    src = np.asarray(inputs["src_mask"])[0, 0, 0].astype(np.float32)  # [S]
    g1, b1 = f32(inputs["n1_g"]), f32(inputs["n1_b"])
    g2, b2 = f32(inputs["n2_g"]), f32(inputs["n2_b"])
    g3, b3 = f32(inputs["n3_g"]), f32(inputs["n3_b"])
    scale = np.float32(1.0 / np.sqrt(DK))

    w = {}
    w["sa_wq"] = bf((g1[:, None] * f32(inputs["sa_wq"])) * scale)
    sa_bq = (b1 @ f32(inputs["sa_wq"]) + f32(inputs["sa_bq"])) * scale
    w["sa_wk"] = bf(g1[:, None] * f32(inputs["sa_wk"]))
    sa_bk = b1 @ f32(inputs["sa_wk"]) + f32(inputs["sa_bk"])
    w["sa_wv"] = bf(g1[:, None] * f32(inputs["sa_wv"]))
    sa_bv = b1 @ f32(inputs["sa_wv"]) + f32(inputs["sa_bv"])
    w["sa_wo"] = bf(inputs["sa_wo"])
    sa_bo = f32(inputs["sa_bo"])
    w["ca_wq"] = bf((g2[:, None] * f32(inputs["ca_wq"])) * scale)
    ca_bq = (b2 @ f32(inputs["ca_wq"]) + f32(inputs["ca_bq"])) * scale
    w["ca_wk"] = bf(inputs["ca_wk"])
    ca_bk = f32(inputs["ca_bk"])
    w["ca_wv"] = bf(inputs["ca_wv"])
    ca_bv = f32(inputs["ca_bv"])
    w["ca_wo"] = bf(inputs["ca_wo"])
    ca_bo = f32(inputs["ca_bo"])
    w["ff_w1"] = bf(g3[:, None] * f32(inputs["ff_w1"]))
    ff_b1 = b3 @ f32(inputs["ff_w1"]) + f32(inputs["ff_b1"])
    w["ff_w2"] = bf(inputs["ff_w2"])
    ff_b2 = f32(inputs["ff_b2"])

    col = lambda b: np.ascontiguousarray(np.asarray(b, np.float32).reshape(-1, 128).T)
    row = lambda b: np.ascontiguousarray(np.asarray(b, np.float32).reshape(1, -1))
    shared = dict(w)
    shared["sa_bq"] = col(sa_bq)
    shared["sa_bk"] = col(sa_bk)
    shared["sa_bo"] = col(sa_bo)
    shared["ca_bq"] = col(ca_bq)
    shared["ca_bk"] = col(ca_bk)
    shared["ca_bo"] = col(ca_bo)
    shared["ff_b2"] = col(ff_b2)
    shared["sa_bv"] = row(sa_bv)
    shared["ca_bv"] = row(ca_bv)
    shared["ff_b1"] = col(ff_b1)
    shared["ones_r"] = np.ones((1, 128), np.float32)
    shared["ones_c"] = np.ones((128, 1), np.float32)

    apply_src_mask = not bool(np.all(src == 1.0))
    if apply_src_mask:
        shared["maskc"] = np.ascontiguousarray(src.reshape(S, 1).astype(BF))

    in_maps = []
    for core in range(8):
        b, c = core // 4, core % 4
        q0 = c * CH
        perm = np.r_[q0:q0 + CH, 0:q0, q0 + CH:S]
        m = dict(shared)
        m["xT"] = np.ascontiguousarray(x[b].T[:, perm])
        m["encT"] = np.ascontiguousarray(enc[b].T.astype(BF))
        m["maskT"] = np.ascontiguousarray(tgt[q0:q0 + CH, :].T[perm, :].astype(BF))
        in_maps.append(m)
    return in_maps, apply_src_mask


def kernel(**inputs):
    from concourse.bass_utils import run_bass_kernel_spmd

    in_maps, apply_src_mask = _prep_host(inputs)
    key = apply_src_mask
    if key not in _CACHE:
        _CACHE[key] = _build(apply_src_mask)
    nc = _CACHE[key]
    res = run_bass_kernel_spmd(nc, in_maps, core_ids=list(range(8)))
    out = np.empty((2, S, D), np.float32)
    for core in range(8):
        b, c = core // 4, core % 4
        out[b, c * CH:(c + 1) * CH, :] = res.results[core]["yT"].T
    return out


# revision 18
# speedup vs baseline: 2.6188x; 1.0105x over previous
"""Trainium2 Bass kernel for a pre-norm transformer decoder layer.

kernel(**inputs) takes the full unsharded inputs of reference.setup_inputs()
and returns the full [2, 2048, 1024] fp32 output.

Sharding: 8 NeuronCores, token-parallel, zero collectives. Core i handles
batch b = i // 4 and query chunk c = i % 4 (512 tokens). Each core computes
the full-batch K/V projections it needs locally (SA K/V from LN1(x) of its
batch; CA K/V from the raw encoder output). The token axis is rolled per
core so its own query chunk sits at positions [0:512) -- one SPMD program,
per-core data only.

Numerics: matmul OPERANDS are bf16 (fp32 runs as two PE passes -- half
throughput); accumulation is always fp32 in PSUM. The LayerNorm statistics,
softmax denominators/reciprocals, biases, and the entire residual stream
stay fp32, so rounding error does not compound across blocks.

Layout is feature-major (xT [D, T]): weights load as lhsT with the
contraction on partitions; no activation transposes anywhere. LayerNorm
stats use ones-column matmuls (partition reductions on PE), softmax runs
without max-subtraction (scores are O(1) here by construction), the mask is
multiplicative 0/1 applied after exp (exact for any mask content), and the
softmax denominator falls out of an appended ones-column in V. gamma/beta
and the attention scale are folded into the weights on the host. K^T and V
are spilled to DRAM scratch (bf16) and streamed back per head.
"""

import sys
sys.path.insert(0, "/opt/trn_rl_repo")

import numpy as np

D = 1024
H = 16
DK = 64
DFF = 4096
S = 2048
CH = 512
EPS = 1e-6
CT = D // 128    # 8 feature tiles
TT = S // 128    # 16 token tiles
FT = DFF // 128  # 32 ff tiles
NG = 2           # score k-tiles per exp/mask group

_CACHE = {}


def _build(apply_src_mask: bool):
    import concourse.bacc as bacc
    import concourse.tile as tile
    from concourse import mybir

    F32 = mybir.dt.float32
    BF16 = mybir.dt.bfloat16
    AF = mybir.ActivationFunctionType
    OP = mybir.AluOpType

    nc = bacc.Bacc("TRN2", target_bir_lowering=False, debug=False)

    xT = nc.dram_tensor("xT", [D, S], F32, kind="ExternalInput")
    encT = nc.dram_tensor("encT", [D, S], BF16, kind="ExternalInput")
    maskT_d = nc.dram_tensor("maskT", [S, CH], BF16, kind="ExternalInput")
    maskc_d = None
    if apply_src_mask:
        maskc_d = nc.dram_tensor("maskc", [S, 1], BF16, kind="ExternalInput")
    w_d = {}
    for nm in ("sa_wq", "sa_wk", "sa_wv", "sa_wo", "ca_wq", "ca_wk", "ca_wv", "ca_wo"):
        w_d[nm] = nc.dram_tensor(nm, [D, D], BF16, kind="ExternalInput")
    w_d["ff_w1"] = nc.dram_tensor("ff_w1", [D, DFF], BF16, kind="ExternalInput")
    w_d["ff_w2"] = nc.dram_tensor("ff_w2", [DFF, D], BF16, kind="ExternalInput")
    bc_d = {}  # bias columns [128, CT] fp32
    for nm in ("sa_bq", "sa_bk", "sa_bo", "ca_bq", "ca_bk", "ca_bo", "ff_b2"):
        bc_d[nm] = nc.dram_tensor(nm, [128, CT], F32, kind="ExternalInput")
    br_d = {}  # bias rows [1, D] fp32 (rank-1 adds on token-major outputs)
    for nm in ("sa_bv", "ca_bv"):
        br_d[nm] = nc.dram_tensor(nm, [1, D], F32, kind="ExternalInput")
    fb1_d = nc.dram_tensor("ff_b1", [128, FT], F32, kind="ExternalInput")
    ones_r_d = nc.dram_tensor("ones_r", [1, 128], F32, kind="ExternalInput")
    ones_c_d = nc.dram_tensor("ones_c", [128, 1], F32, kind="ExternalInput")
    yT = nc.dram_tensor("yT", [D, CH], F32, kind="ExternalOutput")

    with tile.TileContext(nc) as tc:
        with (
            tc.tile_pool(name="const", bufs=1) as constp,
            tc.tile_pool(name="dram", bufs=1, space="DRAM") as dram,
        ):
            t_ones_r = constp.tile([1, 128], F32, tag="ones_r")
            nc.sync.dma_start(t_ones_r[:], ones_r_d[:])
            t_ones_sq = constp.tile([128, 128], F32, tag="ones_sq")
            nc.vector.memset(t_ones_sq[:], 1.0)
            t_ones_c = constp.tile([128, 1], F32, tag="ones_c")
            nc.sync.dma_start(t_ones_c[:], ones_c_d[:])
            t_bc = {}
            for nm, hnd in bc_d.items():
                t_bc[nm] = constp.tile([128, CT], F32, tag=f"b_{nm}", name=f"b_{nm}")
                nc.sync.dma_start(t_bc[nm][:], hnd[:])
            t_br = {}
            for nm, hnd in br_d.items():
                t_br[nm] = constp.tile([1, D], F32, tag=f"b_{nm}", name=f"b_{nm}")
                nc.sync.dma_start(t_br[nm][:], hnd[:])
            t_fb1 = constp.tile([128, FT], F32, tag="b_ff_b1")
            nc.sync.dma_start(t_fb1[:], fb1_d[:])
            t_maskc = None
            if apply_src_mask:
                t_maskc = constp.tile([128, TT], BF16, tag="maskc")
                nc.sync.dma_start(t_maskc[:],
                                  maskc_d.rearrange("(t p) o -> p (t o)", p=128))

            # DRAM scratch (K/V in bf16; residual stream fp32)
            k_sa_scr = dram.tile([D, S], BF16, tag="k_sa")
            v_sa_scr = dram.tile([S, D], BF16, tag="v_sa")
            k_ca_scr = dram.tile([D, S], BF16, tag="k_ca")
            v_ca_scr = dram.tile([S, D], BF16, tag="v_ca")
            x1_scr = dram.tile([D, CH], F32, tag="x1")
            x2_scr = dram.tile([D, CH], F32, tag="x2")

            # ------------------------------------------------------------
            # helpers
            # ------------------------------------------------------------

            def ln_stats(src_dram, ntok, rows_pool):
                """Feature-major fp32 DRAM src [D, ntok] -> (mean, rstd) lists
                of [1, 512] fp32 SBUF tiles in rows_pool."""
                nch = ntok // 512
                mean = [rows_pool.tile([1, 512], F32, tag=f"mean{i}", name=f"mean{i}")
                        for i in range(nch)]
                rstd = [rows_pool.tile([1, 512], F32, tag=f"rstd{i}", name=f"rstd{i}")
                        for i in range(nch)]
                with (
                    tc.tile_pool(name="lns", bufs=2) as sp,
                    tc.tile_pool(name="lnp", bufs=1, space="PSUM") as pp,
                ):
                    s1c = [pp.tile([1, 512], F32, tag=f"s1_{i}", name=f"s1_{i}")
                           for i in range(nch)]
                    s2c = [pp.tile([1, 512], F32, tag=f"s2_{i}", name=f"s2_{i}")
                           for i in range(nch)]
                    for c in range(CT):
                        xc = sp.tile([128, ntok], F32, tag="xs")
                        nc.gpsimd.dma_start(xc[:], src_dram[c * 128:(c + 1) * 128, :])
                        xsq = sp.tile([128, ntok], F32, tag="sq")
                        nc.scalar.activation(xsq[:], xc[:], AF.Square)
                        for ch in range(nch):
                            sl = slice(ch * 512, (ch + 1) * 512)
                            nc.tensor.matmul(s1c[ch][:], t_ones_c[:], xc[:, sl],
                                             start=(c == 0), stop=(c == CT - 1))
                            nc.tensor.matmul(s2c[ch][:], t_ones_c[:], xsq[:, sl],
                                             start=(c == 0), stop=(c == CT - 1))
                    for ch in range(nch):
                        t1 = sp.tile([1, 512], F32, tag="t1")
                        t2 = sp.tile([1, 512], F32, tag="t2")
                        nc.vector.tensor_scalar_mul(mean[ch][:], s1c[ch][:], 1.0 / D)
                        nc.vector.tensor_mul(t1[:], s1c[ch][:], mean[ch][:])
                        nc.vector.tensor_sub(t1[:], s2c[ch][:], t1[:])
                        nc.vector.tensor_scalar_mul(t1[:], t1[:], 1.0 / (D - 1))
                        nc.scalar.activation(t2[:], t1[:], AF.Sqrt)
                        nc.vector.tensor_scalar_add(t2[:], t2[:], EPS)
                        nc.vector.reciprocal(rstd[ch][:], t2[:])
                return mean, rstd

            def proj_block(h, sp, pp, wp, *, half, w_list):
                """Projections for one token half. h: bf16 [128, CT, 1024].

                spec kind 'kT': out feature-major (rhs = h, lhsT = weight),
                evict + bias_col -> bf16 scr [D, S] (or SBUF q when qonly).
                spec kind 'v': out token-major (lhsT = h, rhs = weight),
                rank-1 fp32 bias row, evict -> bf16 scr [S, D]."""
                base = half * 1024
                for spec in w_list:
                    if spec.get("qonly") and half != 0:
                        continue
                    wd = spec["w"]
                    if spec["kind"] == "kT":
                        nch = 1 if spec.get("qonly") else 2
                        for dh in range(2):
                            strips = []
                            for c in range(CT):
                                t = wp.tile([128, 512], BF16, tag="wstr", name="wstr")
                                nc.scalar.dma_start(
                                    t[:], wd[c * 128:(c + 1) * 128,
                                             dh * 512:(dh + 1) * 512])
                                strips.append(t)
                            for dq in range(4):
                                d = dh * 4 + dq
                                for ch in range(nch):
                                    sl = slice(ch * 512, (ch + 1) * 512)
                                    acc = pp.tile([128, 512], F32, tag="mm")
                                    for c in range(CT):
                                        nc.tensor.matmul(
                                            acc[:],
                                            strips[c][:, dq * 128:(dq + 1) * 128],
                                            h[:, c, sl],
                                            start=(c == 0), stop=(c == CT - 1))
                                    if spec.get("qonly"):
                                        nc.vector.tensor_scalar(
                                            spec["out"][:, d, :], acc[:],
                                            spec["bias"][:, d:d + 1], None, OP.add)
                                    else:
                                        ot = sp.tile([128, 512], BF16, tag="kev")
                                        nc.vector.tensor_scalar(
                                            ot[:], acc[:],
                                            spec["bias"][:, d:d + 1], None, OP.add)
                                        nc.sync.dma_start(
                                            spec["out"][d * 128:(d + 1) * 128,
                                                        base + ch * 512:
                                                        base + (ch + 1) * 512],
                                            ot[:])
                    else:  # 'v'
                        for dvc in range(2):
                            bbp = pp.tile([128, 512], F32, tag="mm")
                            nc.tensor.matmul(
                                bbp[:], t_ones_r[:],
                                spec["bias"][:, dvc * 512:(dvc + 1) * 512],
                                start=True, stop=True)
                            bb = sp.tile([128, 512], F32, tag="vbb", bufs=2)
                            nc.vector.tensor_copy(bb[:], bbp[:])
                            strips = []
                            for c in range(CT):
                                t = wp.tile([128, 512], BF16, tag="wstr", name="wstr")
                                nc.scalar.dma_start(
                                    t[:], wd[c * 128:(c + 1) * 128,
                                             dvc * 512:(dvc + 1) * 512])
                                strips.append(t)
                            for tt in range(8):
                                acc = pp.tile([128, 512], F32, tag="mm")
                                for c in range(CT):
                                    nc.tensor.matmul(
                                        acc[:], h[:, c, tt * 128:(tt + 1) * 128],
                                        strips[c][:, :], start=(c == 0),
                                        stop=(c == CT - 1))
                                ot = sp.tile([128, 512], BF16, tag="vev")
                                nc.vector.tensor_add(ot[:], acc[:], bb[:])
                                nc.sync.dma_start(
                                    spec["out"][base + tt * 128:base + (tt + 1) * 128,
                                                dvc * 512:(dvc + 1) * 512],
                                    ot[:])

            def attention(q, k_scr, v_scr, mask_tile, use_maskc, O, sp, pp):
                """q bf16 [128, CT, 512]; K/V streamed bf16 from DRAM scratch.
                Writes O bf16 [64, H, 512] (softmax-normalized per head)."""
                kpair = None
                for h in range(H):
                    dt, pr = h // 2, 64 * (h % 2)
                    if pr == 0:
                        kpair = sp.tile([128, S], BF16, tag="kstr", bufs=3)
                        nc.gpsimd.dma_start(kpair[:],
                                            k_scr[dt * 128:(dt + 1) * 128, :])
                    va = sp.tile([128, TT, 65], BF16, tag="va", bufs=3)
                    nc.gpsimd.dma_start(
                        va[:, :, 0:64],
                        v_scr[:, h * 64:(h + 1) * 64].rearrange(
                            "(t p) d -> p t d", p=128))
                    nc.vector.memset(va[:, :, 64:65], 1.0)
                    grps = []
                    for g in range(TT // NG):
                        sps = pp.tile([128, NG, 512], F32, tag="sc", bufs=2)
                        for j in range(NG):
                            kt = g * NG + j
                            nc.tensor.matmul(
                                sps[:, j, :],
                                kpair[pr:pr + 64, kt * 128:(kt + 1) * 128],
                                q[pr:pr + 64, dt, :],
                                start=True, stop=True)
                        att = sp.tile([128, NG, 512], BF16, tag="att", bufs=12)
                        nc.scalar.activation(att[:], sps[:], AF.Exp)
                        if mask_tile is not None:
                            nc.vector.tensor_mul(att[:], att[:],
                                                 mask_tile[:, g * NG:(g + 1) * NG, :])
                        if use_maskc:
                            for j in range(NG):
                                kt = g * NG + j
                                nc.vector.tensor_scalar(
                                    att[:, j, :], att[:, j, :],
                                    t_maskc[:, kt:kt + 1], None, OP.mult)
                        grps.append(att)
                    avp = pp.tile([65, 512], F32, tag="av")
                    for kt in range(TT):
                        nc.tensor.matmul(avp[:], va[:, kt, :],
                                         grps[kt // NG][:, kt % NG, :],
                                         start=(kt == 0), stop=(kt == TT - 1))
                    rr = sp.tile([65, 512], F32, tag="rr")
                    nc.vector.reciprocal(rr[64:65, :], avp[64:65, :])
                    rbp = pp.tile([64, 512], F32, tag="rb")
                    nc.tensor.matmul(rbp[:], t_ones_sq[64:65, 0:64], rr[64:65, :],
                                     start=True, stop=True)
                    rb = sp.tile([64, 512], F32, tag="rbs")
                    nc.vector.tensor_copy(rb[:], rbp[:])
                    nc.vector.tensor_mul(O[:, h, :], avp[0:64, :], rb[:])

            def out_proj(O, wo_dram, bias_tile, resid_dram, out_dram, sp, pp, wp):
                """out = wo.T @ O + bias_col + resid (fp32), -> out_dram.

                wo strips are loaded per head at partition base 0 so the lhsT
                base matches the O rhs base (matmul requires equal bases)."""
                for oh in range(2):
                    strips = []
                    for h in range(H):
                        t = wp.tile([64, 512], BF16, tag="wstr", name="wstr", bufs=16)
                        nc.scalar.dma_start(t[:], wo_dram[h * 64:(h + 1) * 64,
                                                          oh * 512:(oh + 1) * 512])
                        strips.append(t)
                    for oq in range(4):
                        o = oh * 4 + oq
                        acc = pp.tile([128, 512], F32, tag="mm")
                        for h in range(H):
                            nc.tensor.matmul(
                                acc[:],
                                strips[h][:, oq * 128:(oq + 1) * 128],
                                O[:, h, :], start=(h == 0), stop=(h == H - 1))
                        res = sp.tile([128, 512], F32, tag="res")
                        nc.gpsimd.dma_start(res[:],
                                            resid_dram[o * 128:(o + 1) * 128, 0:512])
                        ot = sp.tile([128, 512], F32, tag="xout")
                        nc.vector.scalar_tensor_tensor(ot[:], acc[:],
                                                       bias_tile[:, o:o + 1],
                                                       res[:], OP.add, OP.add)
                        nc.sync.dma_start(out_dram[o * 128:(o + 1) * 128, :], ot[:])

            def attn_block(src_dram, normalize_src, q_w, q_b, k_w, k_b, v_w, v_br,
                           o_w, o_b, k_scr, v_scr, mask_tile_src, use_maskc,
                           resid_dram, out_dram, q_src_dram, kv_done=False,
                           overlap_emit=None):
                """One full attention block. src_dram: K/V source (fp32 xT for
                SA, bf16 encT for CA). q_src_dram: fp32 LN source for Q when
                not normalize_src (CA: x1_scr)."""
                with tc.tile_pool(name="qkeep", bufs=1) as qkeep:
                    q = qkeep.tile([128, CT, 512], BF16, tag="q")
                    with tc.tile_pool(name="rows", bufs=1) as rows_pool:
                        if normalize_src:
                            mean, rstd = ln_stats(src_dram, S, rows_pool)
                        else:
                            mean, rstd = ln_stats(q_src_dram, CH, rows_pool)
                        with (
                            tc.tile_pool(name="prep", bufs=2) as sp,
                            tc.tile_pool(name="wstr", bufs=16) as wp,
                            tc.tile_pool(name="prepp", bufs=2, space="PSUM") as pp,
                        ):
                            with tc.tile_pool(name="hbuf", bufs=1) as hp:
                                if normalize_src:
                                    # SA: h = LN1(x) bf16, by halves; Q from half 0
                                    h = hp.tile([128, CT, 1024], BF16, tag="h")
                                    for half in range(2):
                                        base = half * 1024
                                        for ch2 in range(2):
                                            chg = half * 2 + ch2
                                            sl = slice(ch2 * 512, (ch2 + 1) * 512)
                                            mb = pp.tile([128, 512], F32, tag="mb")
                                            nc.tensor.matmul(mb[:], t_ones_r[:],
                                                             mean[chg][:],
                                                             start=True, stop=True)
                                            rbb = pp.tile([128, 512], F32, tag="rbb")
                                            nc.tensor.matmul(rbb[:], t_ones_r[:],
                                                             rstd[chg][:],
                                                             start=True, stop=True)
                                            for c in range(CT):
                                                xc = sp.tile([128, 512], F32, tag="xs2")
                                                nc.gpsimd.dma_start(
                                                    xc[:],
                                                    src_dram[c * 128:(c + 1) * 128,
                                                             base + ch2 * 512:
                                                             base + (ch2 + 1) * 512])
                                                nc.vector.tensor_sub(h[:, c, sl],
                                                                     xc[:], mb[:])
                                                nc.vector.tensor_mul(h[:, c, sl],
                                                                     h[:, c, sl],
                                                                     rbb[:])
                                        w_list = [
                                            {"kind": "kT", "w": q_w, "bias": q_b,
                                             "out": q, "qonly": True},
                                            {"kind": "kT", "w": k_w, "bias": k_b,
                                             "out": k_scr},
                                            {"kind": "v", "w": v_w, "bias": v_br,
                                             "out": v_scr},
                                        ]
                                        proj_block(h, sp, pp, wp, half=half,
                                                   w_list=w_list)
                                else:
                                    # CA: Q = LN2(x1) proj; then raw encoder K/V
                                    h2 = hp.tile([128, CT, 1024], BF16, tag="h")
                                    mb = pp.tile([128, 512], F32, tag="mb")
                                    nc.tensor.matmul(mb[:], t_ones_r[:], mean[0][:],
                                                     start=True, stop=True)
                                    rbb = pp.tile([128, 512], F32, tag="rbb")
                                    nc.tensor.matmul(rbb[:], t_ones_r[:], rstd[0][:],
                                                     start=True, stop=True)
                                    for c in range(CT):
                                        xc = sp.tile([128, 512], F32, tag="xs2")
                                        nc.gpsimd.dma_start(
                                            xc[:],
                                            q_src_dram[c * 128:(c + 1) * 128, :])
                                        nc.vector.tensor_sub(h2[:, c, 0:512],
                                                             xc[:], mb[:])
                                        nc.vector.tensor_mul(h2[:, c, 0:512],
                                                             h2[:, c, 0:512], rbb[:])
                                    proj_block(h2, sp, pp, wp, half=0,
                                               w_list=[{"kind": "kT", "w": q_w,
                                                        "bias": q_b, "out": q,
                                                        "qonly": True}])
                                    if not kv_done:
                                        for half in range(2):
                                            base = half * 1024
                                            henc = hp.tile([128, CT, 1024], BF16,
                                                           tag="h", name="henc")
                                            for c in range(CT):
                                                nc.gpsimd.dma_start(
                                                    henc[:, c, :],
                                                    src_dram[c * 128:(c + 1) * 128,
                                                             base:base + 1024])
                                            w_list = [
                                                {"kind": "kT", "w": k_w, "bias": k_b,
                                                 "out": k_scr},
                                                {"kind": "v", "w": v_w, "bias": v_br,
                                                 "out": v_scr},
                                            ]
                                            proj_block(henc, sp, pp, wp, half=half,
                                                       w_list=w_list)
                    # attention + out-proj
                    from contextlib import ExitStack
                    with tc.tile_pool(name="attn_o", bufs=1) as op_, ExitStack() as ovs:
                        O = op_.tile([64, H, 512], BF16, tag="O")
                        if overlap_emit is not None:
                            overlap_emit(ovs)
                        with (
                            tc.tile_pool(name="attn", bufs=6) as sp,
                            tc.tile_pool(name="attnp", bufs=1, space="PSUM") as pp,
                        ):
                            if mask_tile_src is not None:
                                with tc.tile_pool(name="maskp", bufs=1) as mp:
                                    mask_tile = mp.tile([128, TT, 512], BF16, tag="m")
                                    nc.sync.dma_start(
                                        mask_tile[:],
                                        mask_tile_src.rearrange("(t p) q -> p t q",
                                                                p=128))
                                    attention(q, k_scr, v_scr, mask_tile, False,
                                              O, sp, pp)
                            else:
                                attention(q, k_scr, v_scr, None, use_maskc,
                                          O, sp, pp)
                        with (
                            tc.tile_pool(name="oproj", bufs=2) as sp,
                            tc.tile_pool(name="wstro", bufs=1) as wp,
                            tc.tile_pool(name="oprojp", bufs=2, space="PSUM") as pp,
                        ):
                            out_proj(O, o_w, o_b, resid_dram, out_dram, sp, pp, wp)

            # CA K/V production is independent of block 1 -- emit it inside
            # the SA-attention scope so its PE work fills the ACT-bound
            # softmax stretch.
            def ca_kv_overlap(stack):
                csp = stack.enter_context(tc.tile_pool(name="cap", bufs=2))
                cwp = stack.enter_context(tc.tile_pool(name="caw", bufs=16))
                chp = stack.enter_context(tc.tile_pool(name="chb", bufs=1))
                cpp = stack.enter_context(
                    tc.tile_pool(name="capp", bufs=2, space="PSUM"))
                for half in range(2):
                    henc = chp.tile([128, CT, 1024], BF16, tag="h", name="henc")
                    for c in range(CT):
                        nc.gpsimd.dma_start(
                            henc[:, c, :],
                            encT[c * 128:(c + 1) * 128,
                                 half * 1024:(half + 1) * 1024])
                    proj_block(henc, csp, cpp, cwp, half=half, w_list=[
                        {"kind": "kT", "w": w_d["ca_wk"], "bias": t_bc["ca_bk"],
                         "out": k_ca_scr},
                        {"kind": "v", "w": w_d["ca_wv"], "bias": t_br["ca_bv"],
                         "out": v_ca_scr}])

            # ================= Block 1: self-attention =================
            attn_block(xT, True, w_d["sa_wq"], t_bc["sa_bq"], w_d["sa_wk"],
                       t_bc["sa_bk"], w_d["sa_wv"], t_br["sa_bv"], w_d["sa_wo"],
                       t_bc["sa_bo"], k_sa_scr, v_sa_scr, maskT_d, False,
                       xT, x1_scr, None, overlap_emit=ca_kv_overlap)

            # ================= Block 2: cross-attention =================
            attn_block(encT, False, w_d["ca_wq"], t_bc["ca_bq"], w_d["ca_wk"],
                       t_bc["ca_bk"], w_d["ca_wv"], t_br["ca_bv"], w_d["ca_wo"],
                       t_bc["ca_bo"], k_ca_scr, v_ca_scr, None, apply_src_mask,
                       x1_scr, x2_scr, x1_scr, kv_done=True)

            # ================= Block 3: FFN =================
            with tc.tile_pool(name="ffrows", bufs=1) as rows_pool:
                mean3, rstd3 = ln_stats(x2_scr, CH, rows_pool)
                with (
                    tc.tile_pool(name="ffsp", bufs=2) as sp,
                    tc.tile_pool(name="ffw", bufs=4) as wp,
                    tc.tile_pool(name="ffbig", bufs=1) as bigp,
                    tc.tile_pool(name="ffpp", bufs=3, space="PSUM") as pp,
                    tc.tile_pool(name="ffacc", bufs=1, space="PSUM") as accp,
                ):
                    h3 = bigp.tile([128, CT, 512], BF16, tag="h3")
                    mb = pp.tile([128, 512], F32, tag="mm")
                    nc.tensor.matmul(mb[:], t_ones_r[:], mean3[0][:],
                                     start=True, stop=True)
                    rbb = pp.tile([128, 512], F32, tag="mm")
                    nc.tensor.matmul(rbb[:], t_ones_r[:], rstd3[0][:],
                                     start=True, stop=True)
                    for c in range(CT):
                        xc = sp.tile([128, 512], F32, tag="xs3")
                        nc.gpsimd.dma_start(xc[:], x2_scr[c * 128:(c + 1) * 128, :])
                        nc.vector.tensor_sub(h3[:, c, :], xc[:], mb[:])
                        nc.vector.tensor_mul(h3[:, c, :], h3[:, c, :], rbb[:])
                    g = bigp.tile([128, FT, 512], BF16, tag="g")
                    for fh in range(2):
                        strips = []
                        for c in range(CT):
                            t = wp.tile([128, 2048], BF16, tag="w1s", name="w1s",
                                        bufs=8)
                            nc.scalar.dma_start(
                                t[:], w_d["ff_w1"][c * 128:(c + 1) * 128,
                                                   fh * 2048:(fh + 1) * 2048])
                            strips.append(t)
                        for fq in range(16):
                            f = fh * 16 + fq
                            acc = pp.tile([128, 512], F32, tag="mm")
                            for c in range(CT):
                                nc.tensor.matmul(
                                    acc[:], strips[c][:, fq * 128:(fq + 1) * 128],
                                    h3[:, c, :], start=(c == 0), stop=(c == CT - 1))
                            # relu(x + b1) on DVE: (acc + bias) max 0 -> bf16
                            nc.vector.tensor_scalar(g[:, f, :], acc[:],
                                                    t_fb1[:, f:f + 1], 0.0,
                                                    OP.add, OP.max)
                    for oh in range(2):
                        accs = [accp.tile([128, 512], F32, tag=f"acc{i}",
                                          name=f"acc{i}") for i in range(4)]
                        for f in range(FT):
                            w2s = wp.tile([128, 512], BF16, tag="w2s", name="w2s",
                                          bufs=8)
                            nc.scalar.dma_start(
                                w2s[:], w_d["ff_w2"][f * 128:(f + 1) * 128,
                                                     oh * 512:(oh + 1) * 512])
                            for oq in range(4):
                                nc.tensor.matmul(accs[oq][:],
                                                 w2s[:, oq * 128:(oq + 1) * 128],
                                                 g[:, f, :],
                                                 start=(f == 0), stop=(f == FT - 1))
                        for oq in range(4):
                            o = oh * 4 + oq
                            res = sp.tile([128, 512], F32, tag="res3")
                            nc.gpsimd.dma_start(res[:],
                                                x2_scr[o * 128:(o + 1) * 128, :])
                            ot = sp.tile([128, 512], F32, tag="yev")
                            nc.vector.scalar_tensor_tensor(
                                ot[:], accs[oq][:], t_bc["ff_b2"][:, o:o + 1],
                                res[:], OP.add, OP.add)
                            nc.sync.dma_start(yT[o * 128:(o + 1) * 128, :], ot[:])

    nc.compile()
    return nc


def _prep_host(inputs):
    """Host-side folds and per-core data prep."""
    import ml_dtypes
    BF = ml_dtypes.bfloat16
    f32 = lambda a: np.ascontiguousarray(np.asarray(a, np.float32))
    bf = lambda a: np.ascontiguousarray(np.asarray(a, np.float32).astype(BF))
    x = f32(inputs["x"])
    enc = f32(inputs["encoder_output"])
    tgt = np.asarray(inputs["tgt_mask"])[0, 0].astype(np.float32)     # [S, S]
    src = np.asarray(inputs["src_mask"])[0, 0, 0].astype(np.float32)  # [S]
    g1, b1 = f32(inputs["n1_g"]), f32(inputs["n1_b"])
    g2, b2 = f32(inputs["n2_g"]), f32(inputs["n2_b"])
    g3, b3 = f32(inputs["n3_g"]), f32(inputs["n3_b"])
    scale = np.float32(1.0 / np.sqrt(DK))

    w = {}
    w["sa_wq"] = bf((g1[:, None] * f32(inputs["sa_wq"])) * scale)
    sa_bq = (b1 @ f32(inputs["sa_wq"]) + f32(inputs["sa_bq"])) * scale
    w["sa_wk"] = bf(g1[:, None] * f32(inputs["sa_wk"]))
    sa_bk = b1 @ f32(inputs["sa_wk"]) + f32(inputs["sa_bk"])
    w["sa_wv"] = bf(g1[:, None] * f32(inputs["sa_wv"]))
    sa_bv = b1 @ f32(inputs["sa_wv"]) + f32(inputs["sa_bv"])
    w["sa_wo"] = bf(inputs["sa_wo"])
    sa_bo = f32(inputs["sa_bo"])
    w["ca_wq"] = bf((g2[:, None] * f32(inputs["ca_wq"])) * scale)
    ca_bq = (b2 @ f32(inputs["ca_wq"]) + f32(inputs["ca_bq"])) * scale
    w["ca_wk"] = bf(inputs["ca_wk"])
    ca_bk = f32(inputs["ca_bk"])
    w["ca_wv"] = bf(inputs["ca_wv"])
    ca_bv = f32(inputs["ca_bv"])
    w["ca_wo"] = bf(inputs["ca_wo"])
    ca_bo = f32(inputs["ca_bo"])
    w["ff_w1"] = bf(g3[:, None] * f32(inputs["ff_w1"]))
    ff_b1 = b3 @ f32(inputs["ff_w1"]) + f32(inputs["ff_b1"])
    w["ff_w2"] = bf(inputs["ff_w2"])
    ff_b2 = f32(inputs["ff_b2"])

    col = lambda b: np.ascontiguousarray(np.asarray(b, np.float32).reshape(-1, 128).T)
    row = lambda b: np.ascontiguousarray(np.asarray(b, np.float32).reshape(1, -1))
    shared = dict(w)
    shared["sa_bq"] = col(sa_bq)
    shared["sa_bk"] = col(sa_bk)
    shared["sa_bo"] = col(sa_bo)
    shared["ca_bq"] = col(ca_bq)
    shared["ca_bk"] = col(ca_bk)
    shared["ca_bo"] = col(ca_bo)
    shared["ff_b2"] = col(ff_b2)
    shared["sa_bv"] = row(sa_bv)
    shared["ca_bv"] = row(ca_bv)
    shared["ff_b1"] = col(ff_b1)
    shared["ones_r"] = np.ones((1, 128), np.float32)
    shared["ones_c"] = np.ones((128, 1), np.float32)

    apply_src_mask = not bool(np.all(src == 1.0))
    if apply_src_mask:
        shared["maskc"] = np.ascontiguousarray(src.reshape(S, 1).astype(BF))

    in_maps = []
    for core in range(8):
        b, c = core // 4, core % 4
        q0 = c * CH
        perm = np.r_[q0:q0 + CH, 0:q0, q0 + CH:S]
        m = dict(shared)
        m["xT"] = np.ascontiguousarray(x[b].T[:, perm])
        m["encT"] = np.ascontiguousarray(enc[b].T.astype(BF))
        m["maskT"] = np.ascontiguousarray(tgt[q0:q0 + CH, :].T[perm, :].astype(BF))
        in_maps.append(m)
    return in_maps, apply_src_mask


def kernel(**inputs):
    from concourse.bass_utils import run_bass_kernel_spmd

    in_maps, apply_src_mask = _prep_host(inputs)
    key = apply_src_mask
    if key not in _CACHE:
        _CACHE[key] = _build(apply_src_mask)
    nc = _CACHE[key]
    res = run_bass_kernel_spmd(nc, in_maps, core_ids=list(range(8)))
    out = np.empty((2, S, D), np.float32)
    for core in range(8):
        b, c = core // 4, core % 4
        out[b, c * CH:(c + 1) * CH, :] = res.results[core]["yT"].T
    return out
